# revision 15
# baseline (speedup 1.0000x reference)
"""MergedEmbeddingBag forward (sum pooling) on 8 Trainium2 NeuronCores.

Strategy (table-parallel, per sharding hint): core t owns table t, SPMD.

Default (v3) path, fixed-size bags: the f32 table is converted once on
device to a bf16 DRAM copy (SWDGE cast-DMA, overlapped with the pipeline
start).  Gathers then run as one big dma_gather per (4-window group, table
chunk) - chunks of <=25000 rows keep the indices within the int16 HW limit -
writing [occurrence_partition, slot, 128 bf16] tiles.  Pooling is a one-hot
matmul per 128-occurrence slot: DVE builds the one-hot (seg==iota, bf16 out)
from host-precomputed segment labels, PE accumulates single-pass bf16
matmuls into a per-window PSUM tile, ACT copies it out, sync DMAs it back.
All indices/segments are precomputed on the host and streamed in as data.

Measured bottleneck (from NTFF traces): the SWDGE gather pipeline sustains
~2.9 ns per descriptor aggregate across the 4 queue contexts regardless of
descriptor size (256B vs 512B) or call size, so runtime ~= #descriptors x
2.9ns; bf16 halves bytes (engine relief) but not descriptors.
single_packet=True crashes the device; dynamic_dma_scratch_size has no
effect on the cadence.

Fallbacks: general offsets use the v2 f32 path (same one-hot pooling,
per-(window, chunk) gathers); v1 (indirect DMA + DVE strided reduce) kept
for reference.
"""

import sys

sys.path.insert(0, "/opt/trn_rl_repo")

import numpy as np

# Problem geometry (hardcoded per contract; the builder itself is generic).
T = 8
N = 100000
D = 128
B = 16384
TOTAL = 327680
P = 128  # partitions / bags per window
W = B // P  # 128 windows


def _build_program(n_rows, d, n_win, lws, col_ofs, sum_l, g_bufs=6, o_bufs=4):
    """Build the SPMD raw-Bass program (explicit semaphores).

    Pipeline: gpsimd issues SWDGE indirect gathers (bag-major into SBUF),
    DVE does one strided reduce per window, SP (sync) stores pooled tiles.

    n_rows: rows in the (possibly zero-row-extended) weight table
    lws[w]: items per bag in window w (uniform within a window, padded)
    col_ofs[w]: column offset of window w's index block in the idx buffer
    sum_l: total index columns (sum of lws)
    """
    import concourse.bass as bass
    import concourse.mybir as mybir

    lmax = max(lws)
    nc = bass.Bass(num_swdge_queues=4)
    wz = nc.declare_dram_parameter("wz", [n_rows, d], mybir.dt.float32, isOutput=False)
    idx = nc.declare_dram_parameter("idx", [P, sum_l], mybir.dt.int32, isOutput=False)
    out = nc.declare_dram_parameter(
        "out", [n_win * P, d], mybir.dt.float32, isOutput=True
    )

    import contextlib

    with contextlib.ExitStack() as ctx:
        idx_sb = ctx.enter_context(nc.sbuf_tensor([P, sum_l], mybir.dt.int32))
        gbuf = ctx.enter_context(
            nc.sbuf_tensor([P, g_bufs * lmax * d], mybir.dt.float32)
        )
        obuf = ctx.enter_context(nc.sbuf_tensor([P, o_bufs * d], mybir.dt.float32))
        idx_sem = ctx.enter_context(nc.semaphore("idx_sem"))
        # One completion sem per buffer slot: at most one DMA in flight per
        # sem, so ge-16k waits are race-free.
        gsems = [ctx.enter_context(nc.semaphore(f"gsem{i}")) for i in range(g_bufs)]
        ssems = [ctx.enter_context(nc.semaphore(f"ssem{i}")) for i in range(o_bufs)]
        rsem = ctx.enter_context(nc.semaphore("rsem"))
        block = ctx.enter_context(nc.Block())

        def gslot(w):
            s = w % g_bufs
            return gbuf[:, s * lmax * d : s * lmax * d + lws[w] * d]

        def oslot(w):
            s = w % o_bufs
            return obuf[:, s * d : (s + 1) * d]

        @block.sync
        def _(sync):
            sync.dma_start(idx_sb[:], idx[:]).then_inc(idx_sem, 16)
            for w in range(n_win):
                sync.wait_ge(rsem, w + 1)
                sync.dma_start(out[w * P : (w + 1) * P, :], oslot(w)).then_inc(
                    ssems[w % o_bufs], 16
                )
            for lane in range(o_bufs):
                n_l = len(range(lane, n_win, o_bufs))
                if n_l:
                    sync.wait_ge(ssems[lane], 16 * n_l)

        # HW indirect DMA supports exactly one offset per partition per
        # instruction ([P,1] offsets -> [P,elem] dest), so a window of L
        # items takes L gather instructions.  All of window w's gathers
        # inc the window's lane sem; the consumer waits for the lane's
        # cumulative total, which is race-free because the next window on
        # a lane only starts after that wait was consumed (via rsem).
        lane_after = {}
        lane_tot = [0] * g_bufs
        for w in range(n_win):
            lane_tot[w % g_bufs] += 16 * lws[w]
            lane_after[w] = lane_tot[w % g_bufs]

        @block.gpsimd
        def _(g):
            g.wait_ge(idx_sem, 16)
            for w in range(n_win):
                if w >= g_bufs:
                    g.wait_ge(rsem, w - g_bufs + 1)
                base = (w % g_bufs) * (lmax * d)
                for l in range(lws[w]):
                    inst = g.indirect_dma_start(
                        out=gbuf[:, base + l * d : base + (l + 1) * d],
                        out_offset=None,
                        in_=wz[:],
                        in_offset=bass.IndirectOffsetOnAxis(
                            ap=idx_sb[:, col_ofs[w] + l : col_ofs[w] + l + 1],
                            axis=0,
                        ),
                    ).then_inc(gsems[w % g_bufs], 16)
                    # Spread SWDGE desc-gen across all 4 queue contexts —
                    # measured 3.6x throughput vs the single default queue.
                    q = (w * lws[w] + l) % 4
                    if q:
                        inst.ins.queue = f"qPoolDynamic{q}"

        @block.vector
        def _(v):
            for w in range(n_win):
                v.wait_ge(gsems[w % g_bufs], lane_after[w])
                if w >= o_bufs:
                    wp = w - o_bufs
                    v.wait_ge(ssems[wp % o_bufs], 16 * (wp // o_bufs + 1))
                v.reduce_sum(
                    oslot(w),
                    gslot(w).rearrange("p (l e) -> p e l", e=d),
                    axis=mybir.AxisListType.X,
                ).then_inc(rsem, 1)

    return nc


def _plan(indices, offsets, pad_row):
    """Host-side planning: per-table padded bag-major index buffers.

    pad_row: index of the appended all-zeros row (= original table row count).
    Returns (idxbufs [T, P, sum_l] int32, lws, col_ofs, sum_l, need_pad).
    """
    idx64 = np.ascontiguousarray(indices).astype(np.int64)
    off = np.ascontiguousarray(offsets).astype(np.int64)
    t, total = idx64.shape
    b = off.shape[1]
    n_win = b // P

    ends = np.concatenate([off[:, 1:], np.full((t, 1), total, np.int64)], axis=1)
    lens = np.clip(ends - off, 0, None)  # [T, B]

    l_uniform = total // b
    fixed = (
        total == b * l_uniform
        and (lens == l_uniform).all()
        and (off == np.arange(b, dtype=np.int64) * l_uniform).all()
    )

    if fixed:
        lws = [l_uniform] * n_win
        col_ofs = [w * l_uniform for w in range(n_win)]
        sum_l = n_win * l_uniform
        # [t, b, l] -> [t, p, w*L+l]
        bufs = (
            idx64.reshape(t, n_win, P, l_uniform)
            .transpose(0, 2, 1, 3)
            .reshape(t, P, sum_l)
            .astype(np.int32)
        )
        return bufs, lws, col_ofs, sum_l, False

    lws = []
    col_ofs = []
    blocks = []
    need_pad = False
    for w in range(n_win):
        b0 = w * P
        lens_w = lens[:, b0 : b0 + P]  # [T, P]
        lw = max(1, int(lens_w.max()))
        if (lens_w != lw).any():
            need_pad = True
        l_grid = np.arange(lw, dtype=np.int64)
        pos = off[:, b0 : b0 + P, None] + l_grid[None, None, :]  # [T, P, lw]
        valid = l_grid[None, None, :] < lens_w[:, :, None]
        gathered = np.take_along_axis(
            idx64, pos.clip(0, total - 1).reshape(t, -1), axis=1
        ).reshape(t, P, lw)
        blocks.append(np.where(valid, gathered, pad_row).astype(np.int32))
        col_ofs.append(sum(lws))
        lws.append(lw)
    sum_l = sum(lws)
    bufs = np.concatenate(blocks, axis=2)
    return bufs, lws, col_ofs, sum_l, need_pad


def _plan2(indices, offsets, n_rows, chunk=25000):
    """Host planning for the dma_gather path.

    Rows of each 128-bag window are stable-sorted by table chunk
    (idx // chunk) so each run's local indices fit int16.  Runs are padded
    to a multiple of 16 (shared across tables) with dummy index 0; dummy /
    stale positions carry seg = -1 so the one-hot pooling drops them.

    Returns dict with per-table device buffers and the static schedule.
    """
    idx64 = np.ascontiguousarray(indices).astype(np.int64)
    off = np.ascontiguousarray(offsets).astype(np.int64)
    t, total = idx64.shape
    b = off.shape[1]
    n_win = b // P
    n_chunks = -(-n_rows // chunk)
    assert chunk <= 32767

    ends = np.concatenate([off[:, 1:], np.full((t, 1), total, np.int64)], axis=1)
    lens = np.clip(ends - off, 0, None)  # [T, B]

    # Per window, per table: positions and their bag (seg) in window-local
    # terms, sorted by chunk.
    idx_cols = []   # per-(w,c) int16 [T, P16] local indices
    seg_cols = []   # per-(w,slot) f32 [T, 128] segs
    sched = []      # per window: list of (chunk_id, P16, n_slots)
    l_uni = total // b
    uniform = (
        total == b * l_uni
        and (lens == l_uni).all()
        and (off == np.arange(b, dtype=np.int64) * l_uni).all()
    )
    seg_uni = np.repeat(np.arange(P), l_uni)

    for w in range(n_win):
        b0 = w * P
        per_table = []  # (idx_sorted, seg_sorted, chunk_sorted) per table
        for i in range(t):
            if uniform:
                ix = idx64[i, b0 * l_uni : (b0 + P) * l_uni]
                segs = seg_uni
            else:
                ls = lens[i, b0 : b0 + P]
                segs = np.repeat(np.arange(P), ls)
                pos = np.concatenate(
                    [
                        np.arange(off[i, b0 + j], off[i, b0 + j] + ls[j])
                        for j in range(P)
                    ]
                ) if ls.sum() else np.zeros(0, np.int64)
                ix = idx64[i, pos] if len(pos) else np.zeros(0, np.int64)
            c = ix // chunk
            order = np.argsort(c, kind="stable")
            per_table.append((ix[order], segs[order], c[order]))
        wsched = []
        for c in range(n_chunks):
            ns = [int((pt[2] == c).sum()) for pt in per_table]
            mx = max(ns)
            if mx == 0:
                continue
            p16 = -(-mx // 16) * 16
            n_slots = -(-p16 // P)
            ib = np.zeros((t, p16), np.int16)
            sb = np.full((t, n_slots * P), -1.0, np.float32)
            for i in range(t):
                sel = per_table[i][2] == c
                k = ns[i]
                ib[i, :k] = (per_table[i][0][sel] - c * chunk).astype(np.int16)
                sb[i, :k] = per_table[i][1][sel].astype(np.float32)
            idx_cols.append(ib)
            seg_cols.append(sb)
            wsched.append((c, p16, n_slots))
        if not wsched:
            # Empty window: one dummy run so the psum still gets written
            # (with zeros) before the copy-out.
            idx_cols.append(np.zeros((t, 16), np.int16))
            seg_cols.append(np.full((t, P), -1.0, np.float32))
            wsched.append((0, 16, 1))
        sched.append(wsched)

    # Device idx buffer: wrapped [16, cols] replicated to 128 partitions.
    iparts = []
    for ib in idx_cols:
        t_, p16 = ib.shape
        iparts.append(ib.reshape(t_, p16 // 16, 16).transpose(0, 2, 1))
    idxbuf16 = np.concatenate(iparts, axis=2)  # [T, 16, IC]
    idxbuf = np.tile(idxbuf16, (1, 8, 1))  # [T, 128, IC]
    # Device seg buffer: [T, 128, n_slots_total] (seg of (partition, slot)).
    sparts = [sb.reshape(t, -1, P).transpose(0, 2, 1) for sb in seg_cols]
    segbuf = np.concatenate(sparts, axis=2)
    iota = np.tile(np.arange(P, dtype=np.float32)[None, :], (P, 1))
    return {
        "sched": sched,
        "idxbuf": np.ascontiguousarray(idxbuf),
        "segbuf": np.ascontiguousarray(segbuf),
        "iota": iota,
        "chunk": chunk,
    }


def _build_program2(n_rows, d, plan, g_bufs=4, oh_bufs=6, p_bufs=2, o_bufs=4):
    """dma_gather + one-hot-matmul pooling program (raw Bass)."""
    import contextlib

    import concourse.bass as bass
    import concourse.mybir as mybir
    from concourse import library_config

    sched = plan["sched"]
    chunk = plan["chunk"]
    n_win = len(sched)
    ic = plan["idxbuf"].shape[2]
    sc = plan["segbuf"].shape[2]

    # Static per-window derived counts.
    slots_per_win = [sum(ns for _, _, ns in ws) for ws in sched]
    g_per_win = [len(ws) for ws in sched]
    smax = max(slots_per_win)
    mm_after = np.cumsum(slots_per_win)  # matmuls (= slots) completed after w
    lane_after = {}
    lane_tot = [0] * g_bufs
    for w in range(n_win):
        lane_tot[w % g_bufs] += 16 * g_per_win[w]
        lane_after[w] = lane_tot[w % g_bufs]

    nc = bass.Bass(num_swdge_queues=4)
    wz = nc.declare_dram_parameter("wz", [n_rows, d], mybir.dt.float32, isOutput=False)
    idx = nc.declare_dram_parameter("idx", [P, ic], mybir.dt.int16, isOutput=False)
    seg = nc.declare_dram_parameter("seg", [P, sc], mybir.dt.float32, isOutput=False)
    iota = nc.declare_dram_parameter("iota", [P, P], mybir.dt.float32, isOutput=False)
    out = nc.declare_dram_parameter(
        "out", [n_win * P, d], mybir.dt.float32, isOutput=True
    )

    with contextlib.ExitStack() as ctx:
        idx_sb = ctx.enter_context(nc.sbuf_tensor([P, ic], mybir.dt.int16))
        seg_sb = ctx.enter_context(nc.sbuf_tensor([P, sc], mybir.dt.float32))
        iota_sb = ctx.enter_context(nc.sbuf_tensor([P, P], mybir.dt.float32))
        gbuf = ctx.enter_context(
            nc.sbuf_tensor([P, g_bufs * smax * d], mybir.dt.float32)
        )
        ohbuf = ctx.enter_context(nc.sbuf_tensor([P, oh_bufs * P], mybir.dt.float32))
        obuf = ctx.enter_context(nc.sbuf_tensor([P, o_bufs * d], mybir.dt.float32))
        psums = [
            ctx.enter_context(nc.psum_tensor(f"ps{i}", [P, d], mybir.dt.float32))
            for i in range(p_bufs)
        ]
        in_sem = ctx.enter_context(nc.semaphore("in_sem"))
        zsem = ctx.enter_context(nc.semaphore("zsem"))
        gsems = [ctx.enter_context(nc.semaphore(f"gsem{i}")) for i in range(g_bufs)]
        ohsem = ctx.enter_context(nc.semaphore("ohsem"))
        mmsem = ctx.enter_context(nc.semaphore("mmsem"))
        csem = ctx.enter_context(nc.semaphore("csem"))
        ssems = [ctx.enter_context(nc.semaphore(f"ssem{i}")) for i in range(o_bufs)]
        block = ctx.enter_context(nc.Block())

        @block.sync
        def _(sync):
            sync.dma_start(idx_sb[:], idx[:]).then_inc(in_sem, 16)
            sync.dma_start(seg_sb[:], seg[:]).then_inc(in_sem, 16)
            sync.dma_start(iota_sb[:], iota[:]).then_inc(in_sem, 16)
            for w in range(n_win):
                sync.wait_ge(csem, w + 1)
                sync.dma_start(
                    out[w * P : (w + 1) * P, :],
                    obuf[:, (w % o_bufs) * d : (w % o_bufs + 1) * d],
                ).then_inc(ssems[w % o_bufs], 16)
            for lane in range(o_bufs):
                n_l = len(range(lane, n_win, o_bufs))
                if n_l:
                    sync.wait_ge(ssems[lane], 16 * n_l)

        @block.gpsimd
        def _(g):
            g.load_library(library_config.mlp)
            # First-ever use of gbuf: ensure finite contents so one-hot
            # zero-columns can't turn stale NaNs into NaN outputs.
            g.memset(gbuf[:], 0.0).then_inc(zsem, 1)
            g.wait_ge(zsem, 1)
            g.wait_ge(in_sem, 48)
            reg_ctx = g.register("ni_reg")
            ni = reg_ctx.__enter__()
            icol = 0
            for w in range(n_win):
                if w >= g_bufs:
                    g.wait_ge(mmsem, int(mm_after[w - g_bufs]))
                base = (w % g_bufs) * (smax * d)
                sofs = 0
                for c, p16, n_slots in sched[w]:
                    g.reg_mov(ni, p16)
                    g.dma_gather(
                        out_ap=gbuf[
                            :, base + sofs * d : base + (sofs + n_slots) * d
                        ].rearrange("p (s e) -> p s e", e=d),
                        in_ap=wz[c * chunk : min((c + 1) * chunk, n_rows), :],
                        idxs_ap=idx_sb[:, icol : icol + p16 // 16],
                        num_idxs=p16,
                        num_idxs_reg=ni,
                        elem_size=d,
                        single_packet=False,
                        queue_num=w % g_bufs % 4,
                    ).then_inc(gsems[w % g_bufs], 16)
                    icol += p16 // 16
                    sofs += n_slots

        @block.vector
        def _(v):
            v.wait_ge(in_sem, 48)
            j = 0  # global slot index
            for w in range(n_win):
                for s in range(slots_per_win[w]):
                    if j >= oh_bufs:
                        v.wait_ge(mmsem, j - oh_bufs + 1)
                    v.tensor_tensor(
                        out=ohbuf[:, (j % oh_bufs) * P : (j % oh_bufs + 1) * P],
                        in0=seg_sb[:, j : j + 1].to_broadcast([P, P]),
                        in1=iota_sb[:],
                        op=mybir.AluOpType.is_equal,
                    ).then_inc(ohsem, 1)
                    j += 1

        @block.tensor
        def _(pe):
            pe.wait_ge(zsem, 1)
            j = 0
            for w in range(n_win):
                base = (w % g_bufs) * (smax * d)
                pe.wait_ge(gsems[w % g_bufs], lane_after[w])
                if w >= p_bufs:
                    pe.wait_ge(csem, w - p_bufs + 1)
                ns = slots_per_win[w]
                for s in range(ns):
                    pe.wait_ge(ohsem, j + 1)
                    pe.matmul(
                        psums[w % p_bufs][:],
                        lhsT=ohbuf[:, (j % oh_bufs) * P : (j % oh_bufs + 1) * P],
                        rhs=gbuf[:, base + s * d : base + (s + 1) * d],
                        start=(s == 0),
                        stop=(s == ns - 1),
                    ).then_inc(mmsem, 1)
                    j += 1

        @block.scalar
        def _(a):
            for w in range(n_win):
                a.wait_ge(mmsem, int(mm_after[w]))
                if w >= o_bufs:
                    wp = w - o_bufs
                    a.wait_ge(ssems[wp % o_bufs], 16 * (wp // o_bufs + 1))
                a.copy(
                    obuf[:, (w % o_bufs) * d : (w % o_bufs + 1) * d],
                    psums[w % p_bufs][:],
                ).then_inc(csem, 1)

    return nc


def _plan3(indices, offsets, n_rows, chunk=25000, group=4):
    """Host planning for the v3 (bf16, merged-call) path.

    Fixed-size-bag fast path only (falls back to v2 otherwise).  Windows of
    128 bags; groups of `group` windows; per (group, chunk) one dma_gather
    whose index stream is the concatenation of the group's windows'
    chunk-local occurrences, each window's section padded to a multiple of
    128 (pad idx 0 with seg -1, dropped by the one-hot).
    """
    idx64 = np.ascontiguousarray(indices).astype(np.int64)
    off = np.ascontiguousarray(offsets).astype(np.int64)
    t, total = idx64.shape
    b = off.shape[1]
    n_win = b // P
    n_chunks = -(-n_rows // chunk)
    l_uni = total // b
    if not (
        total == b * l_uni
        and (off == np.arange(b, dtype=np.int64) * l_uni).all()
        and n_win % group == 0
    ):
        return None
    n_grp = n_win // group

    # idx stream order = (g, c, w) [one gather per (g, c)]; seg (one-hot)
    # order = (g, w, c) = tensor/vector consumption order.
    idx_cols = []  # per (g, c): int16 [T, n16cols*16]
    seg_blocks = {}  # (g, c, wl) -> float32 [T, a128]
    sched = []  # per g: list of (c, num_idxs, n_slots, [per-w n_slots])
    bags = np.repeat(np.arange(P), l_uni)  # [P*l_uni] bag of each pos in window

    iw = idx64.reshape(t, n_win, P * l_uni)
    cw = iw // chunk  # chunk of each occurrence
    lw = iw - cw * chunk  # chunk-local index

    for g in range(n_grp):
        gsched = []
        for c in range(n_chunks):
            ibs, wslots = [], []
            for wl in range(group):
                w = g * group + wl
                sel = cw[:, w, :] == c  # [T, P*l]
                cnt = sel.sum(axis=1)  # [T]
                mx = int(cnt.max())
                a128 = max(128, -(-mx // 128) * 128)
                ib = np.zeros((t, a128), np.int16)
                sb = np.full((t, a128), -1.0, np.float32)
                for i in range(t):
                    k = int(cnt[i])
                    ib[i, :k] = lw[i, w, sel[i]].astype(np.int16)
                    sb[i, :k] = bags[sel[i]].astype(np.float32)
                ibs.append(ib)
                seg_blocks[(g, c, wl)] = sb
                wslots.append(a128 // 128)
            ib = np.concatenate(ibs, axis=1)
            num = ib.shape[1]
            gsched.append((c, num, num // 128, wslots))
            idx_cols.append(ib)
        sched.append(gsched)
    # seg columns in consumption order: (g, wl, c)
    seg_cols = [
        seg_blocks[(g, c, wl)]
        for g in range(n_grp)
        for wl in range(group)
        for c in range(n_chunks)
    ]

    # wrap idx int16 into [16, cols] replicated to 128 partitions
    iparts = []
    for ib in idx_cols:
        t_, n = ib.shape
        iparts.append(ib.reshape(t_, n // 16, 16).transpose(0, 2, 1))
    idxbuf = np.tile(np.concatenate(iparts, axis=2), (1, 8, 1))  # [T,128,IC]
    sparts = [sb.reshape(t, -1, P).transpose(0, 2, 1) for sb in seg_cols]
    segbuf = np.concatenate(sparts, axis=2)  # [T, 128, n_slots_tot]
    iota = np.tile(np.arange(P, dtype=np.float32)[None, :], (P, 1))
    return {
        "sched": sched,
        "idxbuf": np.ascontiguousarray(idxbuf),
        "segbuf": np.ascontiguousarray(segbuf),
        "iota": iota,
        "chunk": chunk,
        "group": group,
        "n_chunks": n_chunks,
    }


def _build_program3(
    n_rows, d, plan, g_bufs=2, oh_bufs=8, p_bufs=8, o_bufs=4, single_packet=False
):
    """bf16 convert + merged dma_gather + one-hot matmul pooling (raw Bass)."""
    import contextlib

    import concourse.bass as bass
    import concourse.mybir as mybir
    from concourse import library_config

    sched = plan["sched"]
    chunk = plan["chunk"]
    group = plan["group"]
    n_chunks = plan["n_chunks"]
    n_grp = len(sched)
    n_win = n_grp * group
    ic = plan["idxbuf"].shape[2]
    sc = plan["segbuf"].shape[2]

    # static per-group geometry
    grp_slots = [sum(ns for _, _, ns, _ in gs) for gs in sched]  # slots per group
    smax = max(grp_slots)
    # per (g): matmuls completed after group g (= slots)
    mm_after_grp = np.cumsum(grp_slots)
    # per window: matmul count = sum over chunks of its per-window slots
    win_slots = []
    for g in range(n_grp):
        for wl in range(group):
            win_slots.append(sum(gs[3][wl] for gs in sched[g]))
    mm_after_win = np.cumsum(win_slots)

    nc = bass.Bass(num_swdge_queues=4, dynamic_dma_scratch_size=16384)
    wz = nc.declare_dram_parameter("wz", [n_rows, d], mybir.dt.float32, isOutput=False)
    idx = nc.declare_dram_parameter("idx", [P, ic], mybir.dt.int16, isOutput=False)
    seg = nc.declare_dram_parameter("seg", [P, sc], mybir.dt.float32, isOutput=False)
    iota = nc.declare_dram_parameter("iota", [P, P], mybir.dt.float32, isOutput=False)
    out = nc.declare_dram_parameter(
        "out", [n_win * P, d], mybir.dt.float32, isOutput=True
    )
    wzb = nc.dram_tensor("wzb", [n_rows, d], mybir.dt.bfloat16)

    with contextlib.ExitStack() as ctx:
        idx_sb = ctx.enter_context(nc.sbuf_tensor([P, ic], mybir.dt.int16))
        seg_sb = ctx.enter_context(nc.sbuf_tensor([P, sc], mybir.dt.float32))
        iota_sb = ctx.enter_context(nc.sbuf_tensor([P, P], mybir.dt.float32))
        gbuf = ctx.enter_context(
            nc.sbuf_tensor([P, g_bufs * smax * d], mybir.dt.bfloat16)
        )
        ohbuf = ctx.enter_context(nc.sbuf_tensor([P, oh_bufs * P], mybir.dt.bfloat16))
        obuf = ctx.enter_context(nc.sbuf_tensor([P, o_bufs * d], mybir.dt.float32))
        psums = [
            ctx.enter_context(nc.psum_tensor(f"ps{i}", [P, d], mybir.dt.float32))
            for i in range(p_bufs)
        ]
        in_sem = ctx.enter_context(nc.semaphore("in_sem"))
        zsem = ctx.enter_context(nc.semaphore("zsem"))
        conv_sems = [
            ctx.enter_context(nc.semaphore(f"conv{c}")) for c in range(n_chunks)
        ]
        gsems = [ctx.enter_context(nc.semaphore(f"gsem{c}")) for c in range(n_chunks)]
        ohsem = ctx.enter_context(nc.semaphore("ohsem"))
        mmsem = ctx.enter_context(nc.semaphore("mmsem"))
        csem = ctx.enter_context(nc.semaphore("csem"))
        ssems = [ctx.enter_context(nc.semaphore(f"ssem{i}")) for i in range(o_bufs)]
        block = ctx.enter_context(nc.Block())

        @block.sync
        def _(sync):
            sync.dma_start(idx_sb[:], idx[:]).then_inc(in_sem, 16)
            sync.dma_start(seg_sb[:], seg[:]).then_inc(in_sem, 16)
            sync.dma_start(iota_sb[:], iota[:]).then_inc(in_sem, 16)
            for w in range(n_win):
                sync.wait_ge(csem, w + 1)
                sync.dma_start(
                    out[w * P : (w + 1) * P, :],
                    obuf[:, (w % o_bufs) * d : (w % o_bufs + 1) * d],
                ).then_inc(ssems[w % o_bufs], 16)
            for lane in range(o_bufs):
                n_l = len(range(lane, n_win, o_bufs))
                if n_l:
                    sync.wait_ge(ssems[lane], 16 * n_l)

        @block.gpsimd
        def _(g):
            g.load_library(library_config.mlp)
            g.memset(gbuf[:], 0.0).then_inc(zsem, 1)
            # f32 -> bf16 table conversion, one cast-DMA per chunk, spread
            # across SWDGE queues so conversions drain in parallel
            for c in range(n_chunks):
                lo, hi = c * chunk, min((c + 1) * chunk, n_rows)
                inst = g.dma_start(
                    wzb[lo:hi, :], wz[lo:hi, :], single_packet=False
                ).then_inc(conv_sems[c], 16)
                if c % 4:
                    inst.ins.queue = f"qPoolDynamic{c % 4}"
            g.wait_ge(zsem, 1)
            g.wait_ge(in_sem, 48)
            reg_ctx = g.register("ni_reg")
            ni = reg_ctx.__enter__()
            icol = 0
            for gi in range(n_grp):
                if gi >= g_bufs:
                    g.wait_ge(mmsem, int(mm_after_grp[gi - g_bufs]))
                base = (gi % g_bufs) * (smax * d)
                sofs = 0
                for c, num, n_slots, _ in sched[gi]:
                    if gi == 0:
                        g.wait_ge(conv_sems[c], 16)
                    g.reg_mov(ni, num)
                    g.dma_gather(
                        out_ap=gbuf[
                            :, base + sofs * d : base + (sofs + n_slots) * d
                        ].rearrange("p (s e) -> p s e", e=d),
                        in_ap=wzb[c * chunk : min((c + 1) * chunk, n_rows), :],
                        idxs_ap=idx_sb[:, icol : icol + num // 16],
                        num_idxs=num,
                        num_idxs_reg=ni,
                        elem_size=d,
                        single_packet=single_packet,
                        queue_num=c % 4,
                    ).then_inc(gsems[c], 16)
                    icol += num // 16
                    sofs += n_slots

        @block.vector
        def _(v):
            v.wait_ge(in_sem, 48)
            j = 0
            for gi in range(n_grp):
                for s in range(grp_slots[gi]):
                    if j >= oh_bufs:
                        v.wait_ge(mmsem, j - oh_bufs + 1)
                    v.tensor_tensor(
                        out=ohbuf[:, (j % oh_bufs) * P : (j % oh_bufs + 1) * P],
                        in0=seg_sb[:, j : j + 1].to_broadcast([P, P]),
                        in1=iota_sb[:],
                        op=mybir.AluOpType.is_equal,
                    ).then_inc(ohsem, 1)
                    j += 1

        @block.tensor
        def _(pe):
            pe.wait_ge(zsem, 1)
            j = 0  # matmul index in consumption order (= ohbuf ring index)
            for gi in range(n_grp):
                base = (gi % g_bufs) * (smax * d)
                for c, num, n_slots, wslots in sched[gi]:
                    pe.wait_ge(gsems[c], 16 * (gi + 1))
                # per (chunk, window-local): slot offset in the gather stream
                sec_ofs = []  # [chunk][window-local] -> slot offset
                so = 0
                for c, num, n_slots, wslots in sched[gi]:
                    offs = []
                    for wl in range(group):
                        offs.append(so)
                        so += wslots[wl]
                    sec_ofs.append(offs)
                for wl in range(group):
                    wg = gi * group + wl
                    if wg >= p_bufs:
                        pe.wait_ge(csem, wg - p_bufs + 1)
                    ns_w = win_slots[wg]
                    si = 0
                    for ci, (c, num, n_slots, wslots) in enumerate(sched[gi]):
                        for s in range(wslots[wl]):
                            slot = sec_ofs[ci][wl] + s
                            pe.wait_ge(ohsem, j + 1)
                            pe.matmul(
                                psums[wg % p_bufs][:],
                                lhsT=ohbuf[
                                    :, (j % oh_bufs) * P : (j % oh_bufs + 1) * P
                                ],
                                rhs=gbuf[:, base + slot * d : base + (slot + 1) * d],
                                start=(si == 0),
                                stop=(si == ns_w - 1),
                            ).then_inc(mmsem, 1)
                            si += 1
                            j += 1

        @block.scalar
        def _(a):
            for w in range(n_win):
                a.wait_ge(mmsem, int(mm_after_win[w]))
                if w >= o_bufs:
                    wp = w - o_bufs
                    a.wait_ge(ssems[wp % o_bufs], 16 * (wp // o_bufs + 1))
                a.copy(
                    obuf[:, (w % o_bufs) * d : (w % o_bufs + 1) * d],
                    psums[w % p_bufs][:],
                ).then_inc(csem, 1)

    return nc


def _build_program4(
    n_rows, d, plan, g_bufs=6, oh_bufs=12, p_bufs=4, o_bufs=6
):
    """v4: per-(window,chunk) 16-aligned bf16 gathers + one-hot matmul pooling.

    Same schedule/packing as _build_program2 (minimal descriptor count; slot
    tails hold stale data dropped via seg=-1 one-hot columns), but the table
    is converted once to bf16 in DRAM (SWDGE cast-DMA) so gathers move half
    the bytes and the pooling matmuls are single-pass bf16.  Chunk c's
    gathers ride SWDGE queue c so a window's four calls drain in parallel.
    """
    import contextlib

    import concourse.bass as bass
    import concourse.mybir as mybir
    from concourse import library_config

    sched = plan["sched"]
    chunk = plan["chunk"]
    n_win = len(sched)
    n_chunks = max(c for ws in sched for c, _, _ in ws) + 1
    ic = plan["idxbuf"].shape[2]
    sc = plan["segbuf"].shape[2]

    slots_per_win = [sum(ns for _, _, ns in ws) for ws in sched]
    g_per_win = [len(ws) for ws in sched]
    smax = max(slots_per_win)
    mm_after = np.cumsum(slots_per_win)
    # per (w, c): cumulative gather count on chunk c's queue after window w
    gcount = np.zeros((n_win, n_chunks), np.int64)
    run = [0] * n_chunks
    for w in range(n_win):
        for c, _, _ in sched[w]:
            run[c] += 1
        gcount[w] = run

    nc = bass.Bass(num_swdge_queues=4)
    wz = nc.declare_dram_parameter("wz", [n_rows, d], mybir.dt.float32, isOutput=False)
    idx = nc.declare_dram_parameter("idx", [P, ic], mybir.dt.int16, isOutput=False)
    seg = nc.declare_dram_parameter("seg", [P, sc], mybir.dt.float32, isOutput=False)
    iota = nc.declare_dram_parameter("iota", [P, P], mybir.dt.float32, isOutput=False)
    out = nc.declare_dram_parameter(
        "out", [n_win * P, d], mybir.dt.float32, isOutput=True
    )
    wzb = nc.dram_tensor("wzb", [n_rows, d], mybir.dt.bfloat16)

    with contextlib.ExitStack() as ctx:
        idx_sb = ctx.enter_context(nc.sbuf_tensor([P, ic], mybir.dt.int16))
        seg_sb = ctx.enter_context(nc.sbuf_tensor([P, sc], mybir.dt.float32))
        iota_sb = ctx.enter_context(nc.sbuf_tensor([P, P], mybir.dt.float32))
        gbuf = ctx.enter_context(
            nc.sbuf_tensor([P, g_bufs * smax * d], mybir.dt.bfloat16)
        )
        ohbuf = ctx.enter_context(nc.sbuf_tensor([P, oh_bufs * P], mybir.dt.bfloat16))
        obuf = ctx.enter_context(nc.sbuf_tensor([P, o_bufs * d], mybir.dt.float32))
        psums = [
            ctx.enter_context(nc.psum_tensor(f"ps{i}", [P, d], mybir.dt.float32))
            for i in range(p_bufs)
        ]
        in_sem = ctx.enter_context(nc.semaphore("in_sem"))
        zsem = ctx.enter_context(nc.semaphore("zsem"))
        conv_sems = [
            ctx.enter_context(nc.semaphore(f"conv{c}")) for c in range(n_chunks)
        ]
        gsems = [ctx.enter_context(nc.semaphore(f"gsem{c}")) for c in range(n_chunks)]
        ohsem = ctx.enter_context(nc.semaphore("ohsem"))
        mmsem = ctx.enter_context(nc.semaphore("mmsem"))
        csem = ctx.enter_context(nc.semaphore("csem"))
        ssems = [ctx.enter_context(nc.semaphore(f"ssem{i}")) for i in range(o_bufs)]
        block = ctx.enter_context(nc.Block())

        @block.sync
        def _(sync):
            sync.dma_start(idx_sb[:], idx[:]).then_inc(in_sem, 16)
            sync.dma_start(seg_sb[:], seg[:]).then_inc(in_sem, 16)
            sync.dma_start(iota_sb[:], iota[:]).then_inc(in_sem, 16)
            for w in range(n_win):
                sync.wait_ge(csem, w + 1)
                sync.dma_start(
                    out[w * P : (w + 1) * P, :],
                    obuf[:, (w % o_bufs) * d : (w % o_bufs + 1) * d],
                ).then_inc(ssems[w % o_bufs], 16)
            for lane in range(o_bufs):
                n_l = len(range(lane, n_win, o_bufs))
                if n_l:
                    sync.wait_ge(ssems[lane], 16 * n_l)

        @block.gpsimd
        def _(g):
            g.load_library(library_config.mlp)
            g.memset(gbuf[:], 0.0).then_inc(zsem, 1)
            # convert chunk c on SWDGE queue c%4 so conversions drain in
            # parallel and chunk-c gathers (same queue) queue right behind
            # their own chunk's conversion only.
            for c in range(n_chunks):
                lo, hi = c * chunk, min((c + 1) * chunk, n_rows)
                inst = g.dma_start(wzb[lo:hi, :], wz[lo:hi, :]).then_inc(
                    conv_sems[c], 16
                )
                if c % 4:
                    inst.ins.queue = f"qPoolDynamic{c % 4}"
            g.wait_ge(zsem, 1)
            g.wait_ge(in_sem, 48)
            reg_ctx = g.register("ni_reg")
            ni = reg_ctx.__enter__()
            icol = 0
            waited = set()
            for w in range(n_win):
                if w >= g_bufs:
                    g.wait_ge(mmsem, int(mm_after[w - g_bufs]))
                base = (w % g_bufs) * (smax * d)
                sofs = 0
                for c, p16, n_slots in sched[w]:
                    if c not in waited:
                        g.wait_ge(conv_sems[c], 16)
                        waited.add(c)
                    g.reg_mov(ni, p16)
                    g.dma_gather(
                        out_ap=gbuf[
                            :, base + sofs * d : base + (sofs + n_slots) * d
                        ].rearrange("p (s e) -> p s e", e=d),
                        in_ap=wzb[c * chunk : min((c + 1) * chunk, n_rows), :],
                        idxs_ap=idx_sb[:, icol : icol + p16 // 16],
                        num_idxs=p16,
                        num_idxs_reg=ni,
                        elem_size=d,
                        single_packet=False,
                        queue_num=c % 4,
                    ).then_inc(gsems[c], 16)
                    icol += p16 // 16
                    sofs += n_slots

        @block.vector
        def _(v):
            v.wait_ge(in_sem, 48)
            j = 0
            for w in range(n_win):
                for s in range(slots_per_win[w]):
                    if j >= oh_bufs:
                        v.wait_ge(mmsem, j - oh_bufs + 1)
                    v.tensor_tensor(
                        out=ohbuf[:, (j % oh_bufs) * P : (j % oh_bufs + 1) * P],
                        in0=seg_sb[:, j : j + 1].to_broadcast([P, P]),
                        in1=iota_sb[:],
                        op=mybir.AluOpType.is_equal,
                    ).then_inc(ohsem, 1)
                    j += 1

        @block.tensor
        def _(pe):
            pe.wait_ge(zsem, 1)
            j = 0
            for w in range(n_win):
                base = (w % g_bufs) * (smax * d)
                for c, _, _ in sched[w]:
                    pe.wait_ge(gsems[c], 16 * int(gcount[w][c]))
                if w >= p_bufs:
                    pe.wait_ge(csem, w - p_bufs + 1)
                ns = slots_per_win[w]
                for s in range(ns):
                    pe.wait_ge(ohsem, j + 1)
                    pe.matmul(
                        psums[w % p_bufs][:],
                        lhsT=ohbuf[:, (j % oh_bufs) * P : (j % oh_bufs + 1) * P],
                        rhs=gbuf[:, base + s * d : base + (s + 1) * d],
                        start=(s == 0),
                        stop=(s == ns - 1),
                    ).then_inc(mmsem, 1)
                    j += 1

        @block.scalar
        def _(a):
            for w in range(n_win):
                a.wait_ge(mmsem, int(mm_after[w]))
                if w >= o_bufs:
                    wp = w - o_bufs
                    a.wait_ge(ssems[wp % o_bufs], 16 * (wp // o_bufs + 1))
                a.copy(
                    obuf[:, (w % o_bufs) * d : (w % o_bufs + 1) * d],
                    psums[w % p_bufs][:],
                ).then_inc(csem, 1)

    return nc


def _build_program5(n_rows, d, plan, g_bufs=4, oh_bufs=6, p_bufs=2, o_bufs=4):
    """v5: exact _build_program2 skeleton, but the table is converted once to
    bf16 in DRAM (SWDGE cast-DMA prologue) and the gathers/one-hots/matmuls
    run in bf16 (single-pass PE)."""
    import contextlib

    import concourse.bass as bass
    import concourse.mybir as mybir
    from concourse import library_config

    sched = plan["sched"]
    chunk = plan["chunk"]
    n_win = len(sched)
    n_chunks = max(c for ws in sched for c, _, _ in ws) + 1
    ic = plan["idxbuf"].shape[2]
    sc = plan["segbuf"].shape[2]

    slots_per_win = [sum(ns for _, _, ns in ws) for ws in sched]
    g_per_win = [len(ws) for ws in sched]
    smax = max(slots_per_win)
    mm_after = np.cumsum(slots_per_win)
    lane_after = {}
    lane_tot = [0] * g_bufs
    for w in range(n_win):
        lane_tot[w % g_bufs] += 16 * g_per_win[w]
        lane_after[w] = lane_tot[w % g_bufs]

    nc = bass.Bass(num_swdge_queues=4)
    wz = nc.declare_dram_parameter("wz", [n_rows, d], mybir.dt.float32, isOutput=False)
    idx = nc.declare_dram_parameter("idx", [P, ic], mybir.dt.int16, isOutput=False)
    seg = nc.declare_dram_parameter("seg", [P, sc], mybir.dt.float32, isOutput=False)
    iota = nc.declare_dram_parameter("iota", [P, P], mybir.dt.float32, isOutput=False)
    out = nc.declare_dram_parameter(
        "out", [n_win * P, d], mybir.dt.float32, isOutput=True
    )
    wzb = nc.dram_tensor("wzb", [n_rows, d], mybir.dt.bfloat16)

    with contextlib.ExitStack() as ctx:
        idx_sb = ctx.enter_context(nc.sbuf_tensor([P, ic], mybir.dt.int16))
        seg_sb = ctx.enter_context(nc.sbuf_tensor([P, sc], mybir.dt.float32))
        iota_sb = ctx.enter_context(nc.sbuf_tensor([P, P], mybir.dt.float32))
        gbuf = ctx.enter_context(
            nc.sbuf_tensor([P, g_bufs * smax * d], mybir.dt.bfloat16)
        )
        ohbuf = ctx.enter_context(nc.sbuf_tensor([P, oh_bufs * P], mybir.dt.bfloat16))
        obuf = ctx.enter_context(nc.sbuf_tensor([P, o_bufs * d], mybir.dt.float32))
        psums = [
            ctx.enter_context(nc.psum_tensor(f"ps{i}", [P, d], mybir.dt.float32))
            for i in range(p_bufs)
        ]
        in_sem = ctx.enter_context(nc.semaphore("in_sem"))
        zsem = ctx.enter_context(nc.semaphore("zsem"))
        conv_sem = ctx.enter_context(nc.semaphore("conv_sem"))
        gsems = [ctx.enter_context(nc.semaphore(f"gsem{i}")) for i in range(g_bufs)]
        ohsem = ctx.enter_context(nc.semaphore("ohsem"))
        mmsem = ctx.enter_context(nc.semaphore("mmsem"))
        csem = ctx.enter_context(nc.semaphore("csem"))
        ssems = [ctx.enter_context(nc.semaphore(f"ssem{i}")) for i in range(o_bufs)]
        block = ctx.enter_context(nc.Block())

        @block.sync
        def _(sync):
            sync.dma_start(idx_sb[:], idx[:]).then_inc(in_sem, 16)
            sync.dma_start(seg_sb[:], seg[:]).then_inc(in_sem, 16)
            sync.dma_start(iota_sb[:], iota[:]).then_inc(in_sem, 16)
            for w in range(n_win):
                sync.wait_ge(csem, w + 1)
                sync.dma_start(
                    out[w * P : (w + 1) * P, :],
                    obuf[:, (w % o_bufs) * d : (w % o_bufs + 1) * d],
                ).then_inc(ssems[w % o_bufs], 16)
            for lane in range(o_bufs):
                n_l = len(range(lane, n_win, o_bufs))
                if n_l:
                    sync.wait_ge(ssems[lane], 16 * n_l)

        @block.gpsimd
        def _(g):
            g.load_library(library_config.mlp)
            g.memset(gbuf[:], 0.0).then_inc(zsem, 1)
            for c in range(n_chunks):
                lo, hi = c * chunk, min((c + 1) * chunk, n_rows)
                g.dma_start(wzb[lo:hi, :], wz[lo:hi, :]).then_inc(conv_sem, 16)
            g.wait_ge(zsem, 1)
            g.wait_ge(in_sem, 48)
            g.wait_ge(conv_sem, 16 * n_chunks)
            reg_ctx = g.register("ni_reg")
            ni = reg_ctx.__enter__()
            icol = 0
            for w in range(n_win):
                if w >= g_bufs:
                    g.wait_ge(mmsem, int(mm_after[w - g_bufs]))
                base = (w % g_bufs) * (smax * d)
                sofs = 0
                for c, p16, n_slots in sched[w]:
                    g.reg_mov(ni, p16)
                    g.dma_gather(
                        out_ap=gbuf[
                            :, base + sofs * d : base + (sofs + n_slots) * d
                        ].rearrange("p (s e) -> p s e", e=d),
                        in_ap=wzb[c * chunk : min((c + 1) * chunk, n_rows), :],
                        idxs_ap=idx_sb[:, icol : icol + p16 // 16],
                        num_idxs=p16,
                        num_idxs_reg=ni,
                        elem_size=d,
                        single_packet=False,
                        queue_num=w % g_bufs % 4,
                    ).then_inc(gsems[w % g_bufs], 16)
                    icol += p16 // 16
                    sofs += n_slots

        @block.vector
        def _(v):
            v.wait_ge(in_sem, 48)
            j = 0
            for w in range(n_win):
                for s in range(slots_per_win[w]):
                    if j >= oh_bufs:
                        v.wait_ge(mmsem, j - oh_bufs + 1)
                    v.tensor_tensor(
                        out=ohbuf[:, (j % oh_bufs) * P : (j % oh_bufs + 1) * P],
                        in0=seg_sb[:, j : j + 1].to_broadcast([P, P]),
                        in1=iota_sb[:],
                        op=mybir.AluOpType.is_equal,
                    ).then_inc(ohsem, 1)
                    j += 1

        @block.tensor
        def _(pe):
            pe.wait_ge(zsem, 1)
            j = 0
            for w in range(n_win):
                base = (w % g_bufs) * (smax * d)
                pe.wait_ge(gsems[w % g_bufs], lane_after[w])
                if w >= p_bufs:
                    pe.wait_ge(csem, w - p_bufs + 1)
                ns = slots_per_win[w]
                for s in range(ns):
                    pe.wait_ge(ohsem, j + 1)
                    pe.matmul(
                        psums[w % p_bufs][:],
                        lhsT=ohbuf[:, (j % oh_bufs) * P : (j % oh_bufs + 1) * P],
                        rhs=gbuf[:, base + s * d : base + (s + 1) * d],
                        start=(s == 0),
                        stop=(s == ns - 1),
                    ).then_inc(mmsem, 1)
                    j += 1

        @block.scalar
        def _(a):
            for w in range(n_win):
                a.wait_ge(mmsem, int(mm_after[w]))
                if w >= o_bufs:
                    wp = w - o_bufs
                    a.wait_ge(ssems[wp % o_bufs], 16 * (wp // o_bufs + 1))
                a.copy(
                    obuf[:, (w % o_bufs) * d : (w % o_bufs + 1) * d],
                    psums[w % p_bufs][:],
                ).then_inc(csem, 1)

    return nc


def _run(weights, indices, offsets, trace=False, v2=True, chunk=None, v3=True):
    from concourse import mybir
    from concourse.bass_utils import run_bass_kernel_spmd

    weights = np.ascontiguousarray(np.asarray(weights), dtype=np.float32)
    t, n, d = weights.shape

    if v3:
        try:
            chunk4 = chunk
            if chunk4 is None:
                chunk4 = -(-n // max(1, -(-n // 32767)))
            plan = _plan2(indices, offsets, n, chunk=chunk4)
            nc = _build_program4(n, d, plan)
            mybir.codegen_inst_isa_subclasses(nc)
            in_maps = [
                {
                    "wz": weights[i],
                    "idx": np.ascontiguousarray(plan["idxbuf"][i]),
                    "seg": np.ascontiguousarray(plan["segbuf"][i]),
                    "iota": plan["iota"],
                }
                for i in range(t)
            ]
        except Exception:
            in_maps = None
        if in_maps is not None:
            res = run_bass_kernel_spmd(nc, in_maps, list(range(t)), trace=trace)
            out = np.stack([res.results[i]["out"] for i in range(t)], axis=0)
            return out, res

    if v2:
        if chunk is None:
            chunk = -(-n // max(1, -(-n // 32767)))  # even chunks, each <= 32767
        plan = _plan2(indices, offsets, n, chunk=chunk)
        nc = _build_program2(n, d, plan)
        mybir.codegen_inst_isa_subclasses(nc)
        in_maps = [
            {
                "wz": weights[i],
                "idx": np.ascontiguousarray(plan["idxbuf"][i]),
                "seg": np.ascontiguousarray(plan["segbuf"][i]),
                "iota": plan["iota"],
            }
            for i in range(t)
        ]
    else:
        idxbufs, lws, col_ofs, sum_l, need_pad = _plan(indices, offsets, n)
        n_win = np.asarray(offsets).shape[1] // P
        if need_pad:
            wz = np.concatenate([weights, np.zeros((t, 1, d), np.float32)], axis=1)
        else:
            wz = weights
        nc = _build_program(wz.shape[1], d, n_win, lws, col_ofs, sum_l)
        in_maps = [
            {"wz": wz[i], "idx": np.ascontiguousarray(idxbufs[i])} for i in range(t)
        ]
    res = run_bass_kernel_spmd(nc, in_maps, list(range(t)), trace=trace)
    out = np.stack([res.results[i]["out"] for i in range(t)], axis=0)
    return out, res


def kernel(weights, indices, offsets):
    out, _ = _run(weights, indices, offsets)
    return out



# revision 17
# speedup vs baseline: 1.0599x; 1.0599x over previous
"""MergedEmbeddingBag forward (sum pooling) on 8 Trainium2 NeuronCores.

Strategy (table-parallel, per sharding hint): core t owns table t, SPMD.

Default (v3) path, fixed-size bags: the f32 table is converted once on
device to a bf16 DRAM copy (SWDGE cast-DMA, overlapped with the pipeline
start).  Gathers then run as one big dma_gather per (4-window group, table
chunk) - chunks of <=25000 rows keep the indices within the int16 HW limit -
writing [occurrence_partition, slot, 128 bf16] tiles.  Pooling is a one-hot
matmul per 128-occurrence slot: DVE builds the one-hot (seg==iota, bf16 out)
from host-precomputed segment labels, PE accumulates single-pass bf16
matmuls into a per-window PSUM tile, ACT copies it out, sync DMAs it back.
All indices/segments are precomputed on the host and streamed in as data.

Measured bottleneck (from NTFF traces): the SWDGE gather pipeline sustains
~2.9 ns per descriptor aggregate across the 4 queue contexts regardless of
descriptor size (256B vs 512B) or call size, so runtime ~= #descriptors x
2.9ns; bf16 halves bytes (engine relief) but not descriptors.
single_packet=True crashes the device; dynamic_dma_scratch_size has no
effect on the cadence.

Fallbacks: general offsets use the v2 f32 path (same one-hot pooling,
per-(window, chunk) gathers); v1 (indirect DMA + DVE strided reduce) kept
for reference.
"""

import sys

sys.path.insert(0, "/opt/trn_rl_repo")

import numpy as np

# Problem geometry (hardcoded per contract; the builder itself is generic).
T = 8
N = 100000
D = 128
B = 16384
TOTAL = 327680
P = 128  # partitions / bags per window
W = B // P  # 128 windows


def _build_program(n_rows, d, n_win, lws, col_ofs, sum_l, g_bufs=6, o_bufs=4):
    """Build the SPMD raw-Bass program (explicit semaphores).

    Pipeline: gpsimd issues SWDGE indirect gathers (bag-major into SBUF),
    DVE does one strided reduce per window, SP (sync) stores pooled tiles.

    n_rows: rows in the (possibly zero-row-extended) weight table
    lws[w]: items per bag in window w (uniform within a window, padded)
    col_ofs[w]: column offset of window w's index block in the idx buffer
    sum_l: total index columns (sum of lws)
    """
    import concourse.bass as bass
    import concourse.mybir as mybir

    lmax = max(lws)
    nc = bass.Bass(num_swdge_queues=4)
    wz = nc.declare_dram_parameter("wz", [n_rows, d], mybir.dt.float32, isOutput=False)
    idx = nc.declare_dram_parameter("idx", [P, sum_l], mybir.dt.int32, isOutput=False)
    out = nc.declare_dram_parameter(
        "out", [n_win * P, d], mybir.dt.float32, isOutput=True
    )

    import contextlib

    with contextlib.ExitStack() as ctx:
        idx_sb = ctx.enter_context(nc.sbuf_tensor([P, sum_l], mybir.dt.int32))
        gbuf = ctx.enter_context(
            nc.sbuf_tensor([P, g_bufs * lmax * d], mybir.dt.float32)
        )
        obuf = ctx.enter_context(nc.sbuf_tensor([P, o_bufs * d], mybir.dt.float32))
        idx_sem = ctx.enter_context(nc.semaphore("idx_sem"))
        # One completion sem per buffer slot: at most one DMA in flight per
        # sem, so ge-16k waits are race-free.
        gsems = [ctx.enter_context(nc.semaphore(f"gsem{i}")) for i in range(g_bufs)]
        ssems = [ctx.enter_context(nc.semaphore(f"ssem{i}")) for i in range(o_bufs)]
        rsem = ctx.enter_context(nc.semaphore("rsem"))
        block = ctx.enter_context(nc.Block())

        def gslot(w):
            s = w % g_bufs
            return gbuf[:, s * lmax * d : s * lmax * d + lws[w] * d]

        def oslot(w):
            s = w % o_bufs
            return obuf[:, s * d : (s + 1) * d]

        @block.sync
        def _(sync):
            sync.dma_start(idx_sb[:], idx[:]).then_inc(idx_sem, 16)
            for w in range(n_win):
                sync.wait_ge(rsem, w + 1)
                sync.dma_start(out[w * P : (w + 1) * P, :], oslot(w)).then_inc(
                    ssems[w % o_bufs], 16
                )
            for lane in range(o_bufs):
                n_l = len(range(lane, n_win, o_bufs))
                if n_l:
                    sync.wait_ge(ssems[lane], 16 * n_l)

        # HW indirect DMA supports exactly one offset per partition per
        # instruction ([P,1] offsets -> [P,elem] dest), so a window of L
        # items takes L gather instructions.  All of window w's gathers
        # inc the window's lane sem; the consumer waits for the lane's
        # cumulative total, which is race-free because the next window on
        # a lane only starts after that wait was consumed (via rsem).
        lane_after = {}
        lane_tot = [0] * g_bufs
        for w in range(n_win):
            lane_tot[w % g_bufs] += 16 * lws[w]
            lane_after[w] = lane_tot[w % g_bufs]

        @block.gpsimd
        def _(g):
            g.wait_ge(idx_sem, 16)
            for w in range(n_win):
                if w >= g_bufs:
                    g.wait_ge(rsem, w - g_bufs + 1)
                base = (w % g_bufs) * (lmax * d)
                for l in range(lws[w]):
                    inst = g.indirect_dma_start(
                        out=gbuf[:, base + l * d : base + (l + 1) * d],
                        out_offset=None,
                        in_=wz[:],
                        in_offset=bass.IndirectOffsetOnAxis(
                            ap=idx_sb[:, col_ofs[w] + l : col_ofs[w] + l + 1],
                            axis=0,
                        ),
                    ).then_inc(gsems[w % g_bufs], 16)
                    # Spread SWDGE desc-gen across all 4 queue contexts —
                    # measured 3.6x throughput vs the single default queue.
                    q = (w * lws[w] + l) % 4
                    if q:
                        inst.ins.queue = f"qPoolDynamic{q}"

        @block.vector
        def _(v):
            for w in range(n_win):
                v.wait_ge(gsems[w % g_bufs], lane_after[w])
                if w >= o_bufs:
                    wp = w - o_bufs
                    v.wait_ge(ssems[wp % o_bufs], 16 * (wp // o_bufs + 1))
                v.reduce_sum(
                    oslot(w),
                    gslot(w).rearrange("p (l e) -> p e l", e=d),
                    axis=mybir.AxisListType.X,
                ).then_inc(rsem, 1)

    return nc


def _plan(indices, offsets, pad_row):
    """Host-side planning: per-table padded bag-major index buffers.

    pad_row: index of the appended all-zeros row (= original table row count).
    Returns (idxbufs [T, P, sum_l] int32, lws, col_ofs, sum_l, need_pad).
    """
    idx64 = np.ascontiguousarray(indices).astype(np.int64)
    off = np.ascontiguousarray(offsets).astype(np.int64)
    t, total = idx64.shape
    b = off.shape[1]
    n_win = b // P

    ends = np.concatenate([off[:, 1:], np.full((t, 1), total, np.int64)], axis=1)
    lens = np.clip(ends - off, 0, None)  # [T, B]

    l_uniform = total // b
    fixed = (
        total == b * l_uniform
        and (lens == l_uniform).all()
        and (off == np.arange(b, dtype=np.int64) * l_uniform).all()
    )

    if fixed:
        lws = [l_uniform] * n_win
        col_ofs = [w * l_uniform for w in range(n_win)]
        sum_l = n_win * l_uniform
        # [t, b, l] -> [t, p, w*L+l]
        bufs = (
            idx64.reshape(t, n_win, P, l_uniform)
            .transpose(0, 2, 1, 3)
            .reshape(t, P, sum_l)
            .astype(np.int32)
        )
        return bufs, lws, col_ofs, sum_l, False

    lws = []
    col_ofs = []
    blocks = []
    need_pad = False
    for w in range(n_win):
        b0 = w * P
        lens_w = lens[:, b0 : b0 + P]  # [T, P]
        lw = max(1, int(lens_w.max()))
        if (lens_w != lw).any():
            need_pad = True
        l_grid = np.arange(lw, dtype=np.int64)
        pos = off[:, b0 : b0 + P, None] + l_grid[None, None, :]  # [T, P, lw]
        valid = l_grid[None, None, :] < lens_w[:, :, None]
        gathered = np.take_along_axis(
            idx64, pos.clip(0, total - 1).reshape(t, -1), axis=1
        ).reshape(t, P, lw)
        blocks.append(np.where(valid, gathered, pad_row).astype(np.int32))
        col_ofs.append(sum(lws))
        lws.append(lw)
    sum_l = sum(lws)
    bufs = np.concatenate(blocks, axis=2)
    return bufs, lws, col_ofs, sum_l, need_pad


def _plan2(indices, offsets, n_rows, chunk=25000):
    """Host planning for the dma_gather path.

    Rows of each 128-bag window are stable-sorted by table chunk
    (idx // chunk) so each run's local indices fit int16.  Runs are padded
    to a multiple of 16 (shared across tables) with dummy index 0; dummy /
    stale positions carry seg = -1 so the one-hot pooling drops them.

    Returns dict with per-table device buffers and the static schedule.
    """
    idx64 = np.ascontiguousarray(indices).astype(np.int64)
    off = np.ascontiguousarray(offsets).astype(np.int64)
    t, total = idx64.shape
    b = off.shape[1]
    n_win = b // P
    n_chunks = -(-n_rows // chunk)
    assert chunk <= 32767

    ends = np.concatenate([off[:, 1:], np.full((t, 1), total, np.int64)], axis=1)
    lens = np.clip(ends - off, 0, None)  # [T, B]

    # Per window, per table: positions and their bag (seg) in window-local
    # terms, sorted by chunk.
    idx_cols = []   # per-(w,c) int16 [T, P16] local indices
    seg_cols = []   # per-(w,slot) f32 [T, 128] segs
    sched = []      # per window: list of (chunk_id, P16, n_slots)
    l_uni = total // b
    uniform = (
        total == b * l_uni
        and (lens == l_uni).all()
        and (off == np.arange(b, dtype=np.int64) * l_uni).all()
    )
    seg_uni = np.repeat(np.arange(P), l_uni)

    for w in range(n_win):
        b0 = w * P
        per_table = []  # (idx_sorted, seg_sorted, chunk_sorted) per table
        for i in range(t):
            if uniform:
                ix = idx64[i, b0 * l_uni : (b0 + P) * l_uni]
                segs = seg_uni
            else:
                ls = lens[i, b0 : b0 + P]
                segs = np.repeat(np.arange(P), ls)
                pos = np.concatenate(
                    [
                        np.arange(off[i, b0 + j], off[i, b0 + j] + ls[j])
                        for j in range(P)
                    ]
                ) if ls.sum() else np.zeros(0, np.int64)
                ix = idx64[i, pos] if len(pos) else np.zeros(0, np.int64)
            c = ix // chunk
            order = np.argsort(c, kind="stable")
            per_table.append((ix[order], segs[order], c[order]))
        wsched = []
        for c in range(n_chunks):
            ns = [int((pt[2] == c).sum()) for pt in per_table]
            mx = max(ns)
            if mx == 0:
                continue
            p16 = -(-mx // 16) * 16
            n_slots = -(-p16 // P)
            ib = np.zeros((t, p16), np.int16)
            sb = np.full((t, n_slots * P), -1.0, np.float32)
            for i in range(t):
                sel = per_table[i][2] == c
                k = ns[i]
                ib[i, :k] = (per_table[i][0][sel] - c * chunk).astype(np.int16)
                sb[i, :k] = per_table[i][1][sel].astype(np.float32)
            idx_cols.append(ib)
            seg_cols.append(sb)
            wsched.append((c, p16, n_slots))
        if not wsched:
            # Empty window: one dummy run so the psum still gets written
            # (with zeros) before the copy-out.
            idx_cols.append(np.zeros((t, 16), np.int16))
            seg_cols.append(np.full((t, P), -1.0, np.float32))
            wsched.append((0, 16, 1))
        sched.append(wsched)

    # Device idx buffer: wrapped [16, cols] replicated to 128 partitions.
    iparts = []
    for ib in idx_cols:
        t_, p16 = ib.shape
        iparts.append(ib.reshape(t_, p16 // 16, 16).transpose(0, 2, 1))
    idxbuf16 = np.concatenate(iparts, axis=2)  # [T, 16, IC]
    idxbuf = np.tile(idxbuf16, (1, 8, 1))  # [T, 128, IC]
    # Device seg buffer: [T, 128, n_slots_total] (seg of (partition, slot)).
    sparts = [sb.reshape(t, -1, P).transpose(0, 2, 1) for sb in seg_cols]
    segbuf = np.concatenate(sparts, axis=2)
    iota = np.tile(np.arange(P, dtype=np.float32)[None, :], (P, 1))
    return {
        "sched": sched,
        "idxbuf": np.ascontiguousarray(idxbuf),
        "segbuf": np.ascontiguousarray(segbuf),
        "iota": iota,
        "chunk": chunk,
    }


def _build_program2(n_rows, d, plan, g_bufs=4, oh_bufs=6, p_bufs=2, o_bufs=4):
    """dma_gather + one-hot-matmul pooling program (raw Bass)."""
    import contextlib

    import concourse.bass as bass
    import concourse.mybir as mybir
    from concourse import library_config

    sched = plan["sched"]
    chunk = plan["chunk"]
    n_win = len(sched)
    ic = plan["idxbuf"].shape[2]
    sc = plan["segbuf"].shape[2]

    # Static per-window derived counts.
    slots_per_win = [sum(ns for _, _, ns in ws) for ws in sched]
    g_per_win = [len(ws) for ws in sched]
    smax = max(slots_per_win)
    mm_after = np.cumsum(slots_per_win)  # matmuls (= slots) completed after w
    lane_after = {}
    lane_tot = [0] * g_bufs
    for w in range(n_win):
        lane_tot[w % g_bufs] += 16 * g_per_win[w]
        lane_after[w] = lane_tot[w % g_bufs]

    nc = bass.Bass(num_swdge_queues=4)
    wz = nc.declare_dram_parameter("wz", [n_rows, d], mybir.dt.float32, isOutput=False)
    idx = nc.declare_dram_parameter("idx", [P, ic], mybir.dt.int16, isOutput=False)
    seg = nc.declare_dram_parameter("seg", [P, sc], mybir.dt.float32, isOutput=False)
    iota = nc.declare_dram_parameter("iota", [P, P], mybir.dt.float32, isOutput=False)
    out = nc.declare_dram_parameter(
        "out", [n_win * P, d], mybir.dt.float32, isOutput=True
    )

    with contextlib.ExitStack() as ctx:
        idx_sb = ctx.enter_context(nc.sbuf_tensor([P, ic], mybir.dt.int16))
        seg_sb = ctx.enter_context(nc.sbuf_tensor([P, sc], mybir.dt.float32))
        iota_sb = ctx.enter_context(nc.sbuf_tensor([P, P], mybir.dt.float32))
        gbuf = ctx.enter_context(
            nc.sbuf_tensor([P, g_bufs * smax * d], mybir.dt.float32)
        )
        ohbuf = ctx.enter_context(nc.sbuf_tensor([P, oh_bufs * P], mybir.dt.float32))
        obuf = ctx.enter_context(nc.sbuf_tensor([P, o_bufs * d], mybir.dt.float32))
        psums = [
            ctx.enter_context(nc.psum_tensor(f"ps{i}", [P, d], mybir.dt.float32))
            for i in range(p_bufs)
        ]
        in_sem = ctx.enter_context(nc.semaphore("in_sem"))
        zsem = ctx.enter_context(nc.semaphore("zsem"))
        gsems = [ctx.enter_context(nc.semaphore(f"gsem{i}")) for i in range(g_bufs)]
        ohsem = ctx.enter_context(nc.semaphore("ohsem"))
        mmsem = ctx.enter_context(nc.semaphore("mmsem"))
        csem = ctx.enter_context(nc.semaphore("csem"))
        ssems = [ctx.enter_context(nc.semaphore(f"ssem{i}")) for i in range(o_bufs)]
        block = ctx.enter_context(nc.Block())

        @block.sync
        def _(sync):
            sync.dma_start(idx_sb[:], idx[:]).then_inc(in_sem, 16)
            sync.dma_start(seg_sb[:], seg[:]).then_inc(in_sem, 16)
            sync.dma_start(iota_sb[:], iota[:]).then_inc(in_sem, 16)
            for w in range(n_win):
                sync.wait_ge(csem, w + 1)
                sync.dma_start(
                    out[w * P : (w + 1) * P, :],
                    obuf[:, (w % o_bufs) * d : (w % o_bufs + 1) * d],
                ).then_inc(ssems[w % o_bufs], 16)
            for lane in range(o_bufs):
                n_l = len(range(lane, n_win, o_bufs))
                if n_l:
                    sync.wait_ge(ssems[lane], 16 * n_l)

        @block.gpsimd
        def _(g):
            g.load_library(library_config.mlp)
            # First-ever use of gbuf: ensure finite contents so one-hot
            # zero-columns can't turn stale NaNs into NaN outputs.
            g.memset(gbuf[:], 0.0).then_inc(zsem, 1)
            g.wait_ge(zsem, 1)
            g.wait_ge(in_sem, 48)
            reg_ctx = g.register("ni_reg")
            ni = reg_ctx.__enter__()
            icol = 0
            for w in range(n_win):
                if w >= g_bufs:
                    g.wait_ge(mmsem, int(mm_after[w - g_bufs]))
                base = (w % g_bufs) * (smax * d)
                sofs = 0
                for c, p16, n_slots in sched[w]:
                    g.reg_mov(ni, p16)
                    g.dma_gather(
                        out_ap=gbuf[
                            :, base + sofs * d : base + (sofs + n_slots) * d
                        ].rearrange("p (s e) -> p s e", e=d),
                        in_ap=wz[c * chunk : min((c + 1) * chunk, n_rows), :],
                        idxs_ap=idx_sb[:, icol : icol + p16 // 16],
                        num_idxs=p16,
                        num_idxs_reg=ni,
                        elem_size=d,
                        single_packet=False,
                        queue_num=w % g_bufs % 4,
                    ).then_inc(gsems[w % g_bufs], 16)
                    icol += p16 // 16
                    sofs += n_slots

        @block.vector
        def _(v):
            v.wait_ge(in_sem, 48)
            j = 0  # global slot index
            for w in range(n_win):
                for s in range(slots_per_win[w]):
                    if j >= oh_bufs:
                        v.wait_ge(mmsem, j - oh_bufs + 1)
                    v.tensor_tensor(
                        out=ohbuf[:, (j % oh_bufs) * P : (j % oh_bufs + 1) * P],
                        in0=seg_sb[:, j : j + 1].to_broadcast([P, P]),
                        in1=iota_sb[:],
                        op=mybir.AluOpType.is_equal,
                    ).then_inc(ohsem, 1)
                    j += 1

        @block.tensor
        def _(pe):
            pe.wait_ge(zsem, 1)
            j = 0
            for w in range(n_win):
                base = (w % g_bufs) * (smax * d)
                pe.wait_ge(gsems[w % g_bufs], lane_after[w])
                if w >= p_bufs:
                    pe.wait_ge(csem, w - p_bufs + 1)
                ns = slots_per_win[w]
                for s in range(ns):
                    pe.wait_ge(ohsem, j + 1)
                    pe.matmul(
                        psums[w % p_bufs][:],
                        lhsT=ohbuf[:, (j % oh_bufs) * P : (j % oh_bufs + 1) * P],
                        rhs=gbuf[:, base + s * d : base + (s + 1) * d],
                        start=(s == 0),
                        stop=(s == ns - 1),
                    ).then_inc(mmsem, 1)
                    j += 1

        @block.scalar
        def _(a):
            for w in range(n_win):
                a.wait_ge(mmsem, int(mm_after[w]))
                if w >= o_bufs:
                    wp = w - o_bufs
                    a.wait_ge(ssems[wp % o_bufs], 16 * (wp // o_bufs + 1))
                a.copy(
                    obuf[:, (w % o_bufs) * d : (w % o_bufs + 1) * d],
                    psums[w % p_bufs][:],
                ).then_inc(csem, 1)

    return nc


def _plan3(indices, offsets, n_rows, chunk=25000, group=4):
    """Host planning for the v3 (bf16, merged-call) path.

    Fixed-size-bag fast path only (falls back to v2 otherwise).  Windows of
    128 bags; groups of `group` windows; per (group, chunk) one dma_gather
    whose index stream is the concatenation of the group's windows'
    chunk-local occurrences, each window's section padded to a multiple of
    128 (pad idx 0 with seg -1, dropped by the one-hot).
    """
    idx64 = np.ascontiguousarray(indices).astype(np.int64)
    off = np.ascontiguousarray(offsets).astype(np.int64)
    t, total = idx64.shape
    b = off.shape[1]
    n_win = b // P
    n_chunks = -(-n_rows // chunk)
    l_uni = total // b
    if not (
        total == b * l_uni
        and (off == np.arange(b, dtype=np.int64) * l_uni).all()
        and n_win % group == 0
    ):
        return None
    n_grp = n_win // group

    # idx stream order = (g, c, w) [one gather per (g, c)]; seg (one-hot)
    # order = (g, w, c) = tensor/vector consumption order.
    idx_cols = []  # per (g, c): int16 [T, n16cols*16]
    seg_blocks = {}  # (g, c, wl) -> float32 [T, a128]
    sched = []  # per g: list of (c, num_idxs, n_slots, [per-w n_slots])
    bags = np.repeat(np.arange(P), l_uni)  # [P*l_uni] bag of each pos in window

    iw = idx64.reshape(t, n_win, P * l_uni)
    cw = iw // chunk  # chunk of each occurrence
    lw = iw - cw * chunk  # chunk-local index

    for g in range(n_grp):
        gsched = []
        for c in range(n_chunks):
            ibs, wslots = [], []
            for wl in range(group):
                w = g * group + wl
                sel = cw[:, w, :] == c  # [T, P*l]
                cnt = sel.sum(axis=1)  # [T]
                mx = int(cnt.max())
                a128 = max(128, -(-mx // 128) * 128)
                ib = np.zeros((t, a128), np.int16)
                sb = np.full((t, a128), -1.0, np.float32)
                for i in range(t):
                    k = int(cnt[i])
                    ib[i, :k] = lw[i, w, sel[i]].astype(np.int16)
                    sb[i, :k] = bags[sel[i]].astype(np.float32)
                ibs.append(ib)
                seg_blocks[(g, c, wl)] = sb
                wslots.append(a128 // 128)
            ib = np.concatenate(ibs, axis=1)
            num = ib.shape[1]
            gsched.append((c, num, num // 128, wslots))
            idx_cols.append(ib)
        sched.append(gsched)
    # seg columns in consumption order: (g, wl, c)
    seg_cols = [
        seg_blocks[(g, c, wl)]
        for g in range(n_grp)
        for wl in range(group)
        for c in range(n_chunks)
    ]

    # wrap idx int16 into [16, cols] replicated to 128 partitions
    iparts = []
    for ib in idx_cols:
        t_, n = ib.shape
        iparts.append(ib.reshape(t_, n // 16, 16).transpose(0, 2, 1))
    idxbuf = np.tile(np.concatenate(iparts, axis=2), (1, 8, 1))  # [T,128,IC]
    sparts = [sb.reshape(t, -1, P).transpose(0, 2, 1) for sb in seg_cols]
    segbuf = np.concatenate(sparts, axis=2)  # [T, 128, n_slots_tot]
    iota = np.tile(np.arange(P, dtype=np.float32)[None, :], (P, 1))
    return {
        "sched": sched,
        "idxbuf": np.ascontiguousarray(idxbuf),
        "segbuf": np.ascontiguousarray(segbuf),
        "iota": iota,
        "chunk": chunk,
        "group": group,
        "n_chunks": n_chunks,
    }


def _plan8(indices, offsets, n_rows, chunk=25000, group=8):
    """v8: like _plan3 but per-(group, chunk) calls pad only to 16 indices
    (stale-tail slots handle the rest), and slots may straddle window
    boundaries; straddling slots get one matmul per window with segs masked
    to that window (-1 elsewhere).

    Returns plan with sched per group: list over chunks of
    (c, num_idxs16, n_slots, parts) where parts = list over matmuls of
    (slot, window_local, is_first_for_window, is_last_for_window) resolved
    later; here we return per-part (slot, wl) and per-window first/last
    bookkeeping is done by the builder via win_parts.
    """
    idx64 = np.ascontiguousarray(indices).astype(np.int64)
    off = np.ascontiguousarray(offsets).astype(np.int64)
    t, total = idx64.shape
    b = off.shape[1]
    n_win = b // P
    n_chunks = -(-n_rows // chunk)
    l_uni = total // b
    if not (
        total == b * l_uni
        and (off == np.arange(b, dtype=np.int64) * l_uni).all()
        and n_win % group == 0
    ):
        return None
    n_grp = n_win // group

    bags = np.repeat(np.arange(P), l_uni)
    iw = idx64.reshape(t, n_win, P * l_uni)
    cw = iw // chunk
    lw = iw - cw * chunk

    idx_cols = []  # per (g,c): int16 [T, ceil16]
    seg_cols = []  # per matmul part: f32 [T, 128]
    sched = []  # per g: list of (c, num16, n_slots, parts[(slot, wl)])
    for g in range(n_grp):
        gsched = []
        for c in range(n_chunks):
            sels = []
            cnts = np.zeros((group, t), np.int64)
            for wl in range(group):
                w = g * group + wl
                sel = cw[:, w, :] == c
                sels.append(sel)
                cnts[wl] = sel.sum(axis=1)
            # per-table window section boundaries (padded to the max so the
            # stream layout is shared across tables SPMD)
            secl = cnts.max(axis=1)  # [group] shared section lengths
            starts = np.concatenate([[0], np.cumsum(secl)])
            tot_cols = int(starts[-1])
            num16 = max(16, -(-tot_cols // 16) * 16)
            n_slots = max(1, -(-tot_cols // 128))
            ib = np.zeros((t, num16), np.int16)
            sg = np.full((t, group, n_slots * 128), -1.0, np.float32)
            for i in range(t):
                for wl in range(group):
                    s0 = int(starts[wl])
                    k = int(cnts[wl][i])
                    ib[i, s0 : s0 + k] = lw[i, g * group + wl, sels[wl][i]].astype(
                        np.int16
                    )
                    sg[i, wl, s0 : s0 + k] = bags[sels[wl][i]].astype(np.float32)
            # parts: for each slot, which windows have any live col
            parts = []
            for s in range(n_slots):
                for wl in range(group):
                    lo, hi = int(starts[wl]), int(starts[wl + 1])
                    if lo < (s + 1) * 128 and hi > s * 128:
                        parts.append((s, wl))
                        seg_cols.append(
                            np.ascontiguousarray(sg[:, wl, s * 128 : (s + 1) * 128])
                        )
            gsched.append((c, num16, n_slots, parts))
            idx_cols.append(ib)
        sched.append(gsched)

    iparts = []
    for ib in idx_cols:
        t_, n = ib.shape
        iparts.append(ib.reshape(t_, n // 16, 16).transpose(0, 2, 1))
    idxbuf = np.tile(np.concatenate(iparts, axis=2), (1, 8, 1))
    sparts = [sb.reshape(t, 1, P).transpose(0, 2, 1) for sb in seg_cols]
    segbuf = np.concatenate(sparts, axis=2)
    iota = np.tile(np.arange(P, dtype=np.float32)[None, :], (P, 1))
    return {
        "sched": sched,
        "idxbuf": np.ascontiguousarray(idxbuf),
        "segbuf": np.ascontiguousarray(segbuf),
        "iota": iota,
        "chunk": chunk,
        "group": group,
        "n_chunks": n_chunks,
    }


def _build_program8(n_rows, d, plan, g_bufs=2, oh_bufs=8, p_bufs=8, o_bufs=4):
    """v8 builder: merged 16-padded gathers; boundary slots matmul'd once per
    live window with window-masked segs.  Matmul/one-hot order = stream
    order (g, c, slot, window-part); psum start/stop per window derived from
    each window's global first/last part."""
    import contextlib

    import concourse.bass as bass
    import concourse.mybir as mybir
    from concourse import library_config

    sched = plan["sched"]
    chunk = plan["chunk"]
    group = plan["group"]
    n_chunks = plan["n_chunks"]
    n_grp = len(sched)
    n_win = n_grp * group
    ic = plan["idxbuf"].shape[2]
    sc = plan["segbuf"].shape[2]

    # global matmul (part) list in stream order, with per-window first/last
    all_parts = []  # (g, c_i, slot, wl)
    for g in range(n_grp):
        for c_i, (c, num16, n_slots, parts) in enumerate(sched[g]):
            for (s, wl) in parts:
                all_parts.append((g, c_i, s, wl))
    n_mm = len(all_parts)
    win_first = {}
    win_last = {}
    for j, (g, c_i, s, wl) in enumerate(all_parts):
        wg = g * group + wl
        if wg not in win_first:
            win_first[wg] = j
        win_last[wg] = j
    # matmuls completed after window wg's last part
    mm_after_win = [win_last[w] + 1 for w in range(n_win)]
    grp_slots = [sum(ns for _, _, ns, _ in sched[g]) for g in range(n_grp)]
    smax = max(grp_slots)
    grp_parts = [sum(len(p) for _, _, _, p in sched[g]) for g in range(n_grp)]
    mm_after_grp = np.cumsum(grp_parts)

    nc = bass.Bass(num_swdge_queues=4)
    wz = nc.declare_dram_parameter("wz", [n_rows, d], mybir.dt.float32, isOutput=False)
    idx = nc.declare_dram_parameter("idx", [P, ic], mybir.dt.int16, isOutput=False)
    seg = nc.declare_dram_parameter("seg", [P, sc], mybir.dt.float32, isOutput=False)
    iota = nc.declare_dram_parameter("iota", [P, P], mybir.dt.float32, isOutput=False)
    out = nc.declare_dram_parameter(
        "out", [n_win * P, d], mybir.dt.float32, isOutput=True
    )
    wzb = nc.dram_tensor("wzb", [n_rows, d], mybir.dt.bfloat16)

    with contextlib.ExitStack() as ctx:
        idx_sb = ctx.enter_context(nc.sbuf_tensor([P, ic], mybir.dt.int16))
        seg_sb = ctx.enter_context(nc.sbuf_tensor([P, sc], mybir.dt.float32))
        iota_sb = ctx.enter_context(nc.sbuf_tensor([P, P], mybir.dt.float32))
        gbuf = ctx.enter_context(
            nc.sbuf_tensor([P, g_bufs * smax * d], mybir.dt.bfloat16)
        )
        ohbuf = ctx.enter_context(nc.sbuf_tensor([P, oh_bufs * P], mybir.dt.bfloat16))
        obuf = ctx.enter_context(nc.sbuf_tensor([P, o_bufs * d], mybir.dt.float32))
        psums = [
            ctx.enter_context(nc.psum_tensor(f"ps{i}", [P, d], mybir.dt.float32))
            for i in range(p_bufs)
        ]
        in_sem = ctx.enter_context(nc.semaphore("in_sem"))
        zsem = ctx.enter_context(nc.semaphore("zsem"))
        conv_sems = [
            ctx.enter_context(nc.semaphore(f"conv{c}")) for c in range(n_chunks)
        ]
        gsems = [ctx.enter_context(nc.semaphore(f"gsem{c}")) for c in range(n_chunks)]
        ohsem = ctx.enter_context(nc.semaphore("ohsem"))
        mmsem = ctx.enter_context(nc.semaphore("mmsem"))
        csem = ctx.enter_context(nc.semaphore("csem"))
        ssems = [ctx.enter_context(nc.semaphore(f"ssem{i}")) for i in range(o_bufs)]
        block = ctx.enter_context(nc.Block())

        @block.sync
        def _(sync):
            sync.dma_start(idx_sb[:], idx[:]).then_inc(in_sem, 16)
            sync.dma_start(seg_sb[:], seg[:]).then_inc(in_sem, 16)
            sync.dma_start(iota_sb[:], iota[:]).then_inc(in_sem, 16)
            for w in range(n_win):
                sync.wait_ge(csem, w + 1)
                sync.dma_start(
                    out[w * P : (w + 1) * P, :],
                    obuf[:, (w % o_bufs) * d : (w % o_bufs + 1) * d],
                ).then_inc(ssems[w % o_bufs], 16)
            for lane in range(o_bufs):
                n_l = len(range(lane, n_win, o_bufs))
                if n_l:
                    sync.wait_ge(ssems[lane], 16 * n_l)

        @block.gpsimd
        def _(g):
            g.load_library(library_config.mlp)
            g.memset(gbuf[:], 0.0).then_inc(zsem, 1)
            for c in range(n_chunks):
                lo, hi = c * chunk, min((c + 1) * chunk, n_rows)
                inst = g.dma_start(wzb[lo:hi, :], wz[lo:hi, :]).then_inc(
                    conv_sems[c], 16
                )
                if c % 4:
                    inst.ins.queue = f"qPoolDynamic{c % 4}"
            g.wait_ge(zsem, 1)
            g.wait_ge(in_sem, 48)
            reg_ctx = g.register("ni_reg")
            ni = reg_ctx.__enter__()
            icol = 0
            waited = set()
            for gi in range(n_grp):
                if gi >= g_bufs:
                    g.wait_ge(mmsem, int(mm_after_grp[gi - g_bufs]))
                base = (gi % g_bufs) * (smax * d)
                sofs = 0
                for c, num16, n_slots, parts in sched[gi]:
                    if c not in waited:
                        g.wait_ge(conv_sems[c], 16)
                        waited.add(c)
                    g.reg_mov(ni, num16)
                    g.dma_gather(
                        out_ap=gbuf[
                            :, base + sofs * d : base + (sofs + n_slots) * d
                        ].rearrange("p (s e) -> p s e", e=d),
                        in_ap=wzb[c * chunk : min((c + 1) * chunk, n_rows), :],
                        idxs_ap=idx_sb[:, icol : icol + num16 // 16],
                        num_idxs=num16,
                        num_idxs_reg=ni,
                        elem_size=d,
                        single_packet=False,
                        queue_num=c % 4,
                    ).then_inc(gsems[c], 16)
                    icol += num16 // 16
                    sofs += n_slots

        @block.vector
        def _(v):
            v.wait_ge(in_sem, 48)
            for j in range(n_mm):
                if j >= oh_bufs:
                    v.wait_ge(mmsem, j - oh_bufs + 1)
                v.tensor_tensor(
                    out=ohbuf[:, (j % oh_bufs) * P : (j % oh_bufs + 1) * P],
                    in0=seg_sb[:, j : j + 1].to_broadcast([P, P]),
                    in1=iota_sb[:],
                    op=mybir.AluOpType.is_equal,
                ).then_inc(ohsem, 1)

        @block.tensor
        def _(pe):
            pe.wait_ge(zsem, 1)
            j = 0
            for gi in range(n_grp):
                base = (gi % g_bufs) * (smax * d)
                # slot offset of each chunk-call's region within the group
                call_sofs = []
                so = 0
                for c, num16, n_slots, parts in sched[gi]:
                    call_sofs.append(so)
                    so += n_slots
                for c_i, (c, num16, n_slots, parts) in enumerate(sched[gi]):
                    pe.wait_ge(gsems[c], 16 * (gi + 1))
                    for (s, wl) in parts:
                        wg = gi * group + wl
                        if win_first[wg] == j and wg >= p_bufs:
                            pe.wait_ge(csem, wg - p_bufs + 1)
                        slot = call_sofs[c_i] + s
                        pe.wait_ge(ohsem, j + 1)
                        pe.matmul(
                            psums[wg % p_bufs][:],
                            lhsT=ohbuf[:, (j % oh_bufs) * P : (j % oh_bufs + 1) * P],
                            rhs=gbuf[:, base + slot * d : base + (slot + 1) * d],
                            start=(win_first[wg] == j),
                            stop=(win_last[wg] == j),
                            skip_group_check=True,
                        ).then_inc(mmsem, 1)
                        j += 1

        @block.scalar
        def _(a):
            for w in range(n_win):
                a.wait_ge(mmsem, int(mm_after_win[w]))
                if w >= o_bufs:
                    wp = w - o_bufs
                    a.wait_ge(ssems[wp % o_bufs], 16 * (wp // o_bufs + 1))
                a.copy(
                    obuf[:, (w % o_bufs) * d : (w % o_bufs + 1) * d],
                    psums[w % p_bufs][:],
                ).then_inc(csem, 1)

    return nc


def _build_program3(
    n_rows, d, plan, g_bufs=2, oh_bufs=8, p_bufs=8, o_bufs=4, single_packet=False
):
    """bf16 convert + merged dma_gather + one-hot matmul pooling (raw Bass)."""
    import contextlib

    import concourse.bass as bass
    import concourse.mybir as mybir
    from concourse import library_config

    sched = plan["sched"]
    chunk = plan["chunk"]
    group = plan["group"]
    n_chunks = plan["n_chunks"]
    n_grp = len(sched)
    n_win = n_grp * group
    ic = plan["idxbuf"].shape[2]
    sc = plan["segbuf"].shape[2]

    # static per-group geometry
    grp_slots = [sum(ns for _, _, ns, _ in gs) for gs in sched]  # slots per group
    smax = max(grp_slots)
    # per (g): matmuls completed after group g (= slots)
    mm_after_grp = np.cumsum(grp_slots)
    # per window: matmul count = sum over chunks of its per-window slots
    win_slots = []
    for g in range(n_grp):
        for wl in range(group):
            win_slots.append(sum(gs[3][wl] for gs in sched[g]))
    mm_after_win = np.cumsum(win_slots)

    nc = bass.Bass(num_swdge_queues=4, dynamic_dma_scratch_size=16384)
    wz = nc.declare_dram_parameter("wz", [n_rows, d], mybir.dt.float32, isOutput=False)
    idx = nc.declare_dram_parameter("idx", [P, ic], mybir.dt.int16, isOutput=False)
    seg = nc.declare_dram_parameter("seg", [P, sc], mybir.dt.float32, isOutput=False)
    iota = nc.declare_dram_parameter("iota", [P, P], mybir.dt.float32, isOutput=False)
    out = nc.declare_dram_parameter(
        "out", [n_win * P, d], mybir.dt.float32, isOutput=True
    )
    wzb = nc.dram_tensor("wzb", [n_rows, d], mybir.dt.bfloat16)

    with contextlib.ExitStack() as ctx:
        idx_sb = ctx.enter_context(nc.sbuf_tensor([P, ic], mybir.dt.int16))
        seg_sb = ctx.enter_context(nc.sbuf_tensor([P, sc], mybir.dt.float32))
        iota_sb = ctx.enter_context(nc.sbuf_tensor([P, P], mybir.dt.float32))
        gbuf = ctx.enter_context(
            nc.sbuf_tensor([P, g_bufs * smax * d], mybir.dt.bfloat16)
        )
        ohbuf = ctx.enter_context(nc.sbuf_tensor([P, oh_bufs * P], mybir.dt.bfloat16))
        obuf = ctx.enter_context(nc.sbuf_tensor([P, o_bufs * d], mybir.dt.float32))
        psums = [
            ctx.enter_context(nc.psum_tensor(f"ps{i}", [P, d], mybir.dt.float32))
            for i in range(p_bufs)
        ]
        in_sem = ctx.enter_context(nc.semaphore("in_sem"))
        zsem = ctx.enter_context(nc.semaphore("zsem"))
        conv_sems = [
            ctx.enter_context(nc.semaphore(f"conv{c}")) for c in range(n_chunks)
        ]
        gsems = [ctx.enter_context(nc.semaphore(f"gsem{c}")) for c in range(n_chunks)]
        ohsem = ctx.enter_context(nc.semaphore("ohsem"))
        mmsem = ctx.enter_context(nc.semaphore("mmsem"))
        csem = ctx.enter_context(nc.semaphore("csem"))
        ssems = [ctx.enter_context(nc.semaphore(f"ssem{i}")) for i in range(o_bufs)]
        block = ctx.enter_context(nc.Block())

        @block.sync
        def _(sync):
            sync.dma_start(idx_sb[:], idx[:]).then_inc(in_sem, 16)
            sync.dma_start(seg_sb[:], seg[:]).then_inc(in_sem, 16)
            sync.dma_start(iota_sb[:], iota[:]).then_inc(in_sem, 16)
            for w in range(n_win):
                sync.wait_ge(csem, w + 1)
                sync.dma_start(
                    out[w * P : (w + 1) * P, :],
                    obuf[:, (w % o_bufs) * d : (w % o_bufs + 1) * d],
                ).then_inc(ssems[w % o_bufs], 16)
            for lane in range(o_bufs):
                n_l = len(range(lane, n_win, o_bufs))
                if n_l:
                    sync.wait_ge(ssems[lane], 16 * n_l)

        @block.gpsimd
        def _(g):
            g.load_library(library_config.mlp)
            g.memset(gbuf[:], 0.0).then_inc(zsem, 1)
            # f32 -> bf16 table conversion, one cast-DMA per chunk, spread
            # across SWDGE queues so conversions drain in parallel
            for c in range(n_chunks):
                lo, hi = c * chunk, min((c + 1) * chunk, n_rows)
                inst = g.dma_start(
                    wzb[lo:hi, :], wz[lo:hi, :], single_packet=False
                ).then_inc(conv_sems[c], 16)
                if c % 4:
                    inst.ins.queue = f"qPoolDynamic{c % 4}"
            g.wait_ge(zsem, 1)
            g.wait_ge(in_sem, 48)
            reg_ctx = g.register("ni_reg")
            ni = reg_ctx.__enter__()
            icol = 0
            for gi in range(n_grp):
                if gi >= g_bufs:
                    g.wait_ge(mmsem, int(mm_after_grp[gi - g_bufs]))
                base = (gi % g_bufs) * (smax * d)
                sofs = 0
                for c, num, n_slots, _ in sched[gi]:
                    if gi == 0:
                        g.wait_ge(conv_sems[c], 16)
                    g.reg_mov(ni, num)
                    g.dma_gather(
                        out_ap=gbuf[
                            :, base + sofs * d : base + (sofs + n_slots) * d
                        ].rearrange("p (s e) -> p s e", e=d),
                        in_ap=wzb[c * chunk : min((c + 1) * chunk, n_rows), :],
                        idxs_ap=idx_sb[:, icol : icol + num // 16],
                        num_idxs=num,
                        num_idxs_reg=ni,
                        elem_size=d,
                        single_packet=single_packet,
                        queue_num=c % 4,
                    ).then_inc(gsems[c], 16)
                    icol += num // 16
                    sofs += n_slots

        @block.vector
        def _(v):
            v.wait_ge(in_sem, 48)
            j = 0
            for gi in range(n_grp):
                for s in range(grp_slots[gi]):
                    if j >= oh_bufs:
                        v.wait_ge(mmsem, j - oh_bufs + 1)
                    v.tensor_tensor(
                        out=ohbuf[:, (j % oh_bufs) * P : (j % oh_bufs + 1) * P],
                        in0=seg_sb[:, j : j + 1].to_broadcast([P, P]),
                        in1=iota_sb[:],
                        op=mybir.AluOpType.is_equal,
                    ).then_inc(ohsem, 1)
                    j += 1

        @block.tensor
        def _(pe):
            pe.wait_ge(zsem, 1)
            j = 0  # matmul index in consumption order (= ohbuf ring index)
            for gi in range(n_grp):
                base = (gi % g_bufs) * (smax * d)
                for c, num, n_slots, wslots in sched[gi]:
                    pe.wait_ge(gsems[c], 16 * (gi + 1))
                # per (chunk, window-local): slot offset in the gather stream
                sec_ofs = []  # [chunk][window-local] -> slot offset
                so = 0
                for c, num, n_slots, wslots in sched[gi]:
                    offs = []
                    for wl in range(group):
                        offs.append(so)
                        so += wslots[wl]
                    sec_ofs.append(offs)
                for wl in range(group):
                    wg = gi * group + wl
                    if wg >= p_bufs:
                        pe.wait_ge(csem, wg - p_bufs + 1)
                    ns_w = win_slots[wg]
                    si = 0
                    for ci, (c, num, n_slots, wslots) in enumerate(sched[gi]):
                        for s in range(wslots[wl]):
                            slot = sec_ofs[ci][wl] + s
                            pe.wait_ge(ohsem, j + 1)
                            pe.matmul(
                                psums[wg % p_bufs][:],
                                lhsT=ohbuf[
                                    :, (j % oh_bufs) * P : (j % oh_bufs + 1) * P
                                ],
                                rhs=gbuf[:, base + slot * d : base + (slot + 1) * d],
                                start=(si == 0),
                                stop=(si == ns_w - 1),
                            ).then_inc(mmsem, 1)
                            si += 1
                            j += 1

        @block.scalar
        def _(a):
            for w in range(n_win):
                a.wait_ge(mmsem, int(mm_after_win[w]))
                if w >= o_bufs:
                    wp = w - o_bufs
                    a.wait_ge(ssems[wp % o_bufs], 16 * (wp // o_bufs + 1))
                a.copy(
                    obuf[:, (w % o_bufs) * d : (w % o_bufs + 1) * d],
                    psums[w % p_bufs][:],
                ).then_inc(csem, 1)

    return nc


def _build_program4(
    n_rows, d, plan, g_bufs=6, oh_bufs=12, p_bufs=4, o_bufs=6
):
    """v4: per-(window,chunk) 16-aligned bf16 gathers + one-hot matmul pooling.

    Same schedule/packing as _build_program2 (minimal descriptor count; slot
    tails hold stale data dropped via seg=-1 one-hot columns), but the table
    is converted once to bf16 in DRAM (SWDGE cast-DMA) so gathers move half
    the bytes and the pooling matmuls are single-pass bf16.  Chunk c's
    gathers ride SWDGE queue c so a window's four calls drain in parallel.
    """
    import contextlib

    import concourse.bass as bass
    import concourse.mybir as mybir
    from concourse import library_config

    sched = plan["sched"]
    chunk = plan["chunk"]
    n_win = len(sched)
    n_chunks = max(c for ws in sched for c, _, _ in ws) + 1
    ic = plan["idxbuf"].shape[2]
    sc = plan["segbuf"].shape[2]

    slots_per_win = [sum(ns for _, _, ns in ws) for ws in sched]
    g_per_win = [len(ws) for ws in sched]
    smax = max(slots_per_win)
    mm_after = np.cumsum(slots_per_win)
    # per (w, c): cumulative gather count on chunk c's queue after window w
    gcount = np.zeros((n_win, n_chunks), np.int64)
    run = [0] * n_chunks
    for w in range(n_win):
        for c, _, _ in sched[w]:
            run[c] += 1
        gcount[w] = run

    nc = bass.Bass(num_swdge_queues=4)
    wz = nc.declare_dram_parameter("wz", [n_rows, d], mybir.dt.float32, isOutput=False)
    idx = nc.declare_dram_parameter("idx", [P, ic], mybir.dt.int16, isOutput=False)
    seg = nc.declare_dram_parameter("seg", [P, sc], mybir.dt.float32, isOutput=False)
    iota = nc.declare_dram_parameter("iota", [P, P], mybir.dt.float32, isOutput=False)
    out = nc.declare_dram_parameter(
        "out", [n_win * P, d], mybir.dt.float32, isOutput=True
    )
    wzb = nc.dram_tensor("wzb", [n_rows, d], mybir.dt.bfloat16)

    with contextlib.ExitStack() as ctx:
        idx_sb = ctx.enter_context(nc.sbuf_tensor([P, ic], mybir.dt.int16))
        seg_sb = ctx.enter_context(nc.sbuf_tensor([P, sc], mybir.dt.float32))
        iota_sb = ctx.enter_context(nc.sbuf_tensor([P, P], mybir.dt.float32))
        gbuf = ctx.enter_context(
            nc.sbuf_tensor([P, g_bufs * smax * d], mybir.dt.bfloat16)
        )
        ohbuf = ctx.enter_context(nc.sbuf_tensor([P, oh_bufs * P], mybir.dt.bfloat16))
        obuf = ctx.enter_context(nc.sbuf_tensor([P, o_bufs * d], mybir.dt.float32))
        psums = [
            ctx.enter_context(nc.psum_tensor(f"ps{i}", [P, d], mybir.dt.float32))
            for i in range(p_bufs)
        ]
        in_sem = ctx.enter_context(nc.semaphore("in_sem"))
        zsem = ctx.enter_context(nc.semaphore("zsem"))
        conv_sems = [
            ctx.enter_context(nc.semaphore(f"conv{c}")) for c in range(n_chunks)
        ]
        gsems = [ctx.enter_context(nc.semaphore(f"gsem{c}")) for c in range(n_chunks)]
        ohsem = ctx.enter_context(nc.semaphore("ohsem"))
        mmsem = ctx.enter_context(nc.semaphore("mmsem"))
        csem = ctx.enter_context(nc.semaphore("csem"))
        ssems = [ctx.enter_context(nc.semaphore(f"ssem{i}")) for i in range(o_bufs)]
        block = ctx.enter_context(nc.Block())

        @block.sync
        def _(sync):
            sync.dma_start(idx_sb[:], idx[:]).then_inc(in_sem, 16)
            sync.dma_start(seg_sb[:], seg[:]).then_inc(in_sem, 16)
            sync.dma_start(iota_sb[:], iota[:]).then_inc(in_sem, 16)
            for w in range(n_win):
                sync.wait_ge(csem, w + 1)
                sync.dma_start(
                    out[w * P : (w + 1) * P, :],
                    obuf[:, (w % o_bufs) * d : (w % o_bufs + 1) * d],
                ).then_inc(ssems[w % o_bufs], 16)
            for lane in range(o_bufs):
                n_l = len(range(lane, n_win, o_bufs))
                if n_l:
                    sync.wait_ge(ssems[lane], 16 * n_l)

        @block.gpsimd
        def _(g):
            g.load_library(library_config.mlp)
            g.memset(gbuf[:], 0.0).then_inc(zsem, 1)
            # convert chunk c on SWDGE queue c%4 so conversions drain in
            # parallel and chunk-c gathers (same queue) queue right behind
            # their own chunk's conversion only.
            for c in range(n_chunks):
                lo, hi = c * chunk, min((c + 1) * chunk, n_rows)
                inst = g.dma_start(wzb[lo:hi, :], wz[lo:hi, :]).then_inc(
                    conv_sems[c], 16
                )
                if c % 4:
                    inst.ins.queue = f"qPoolDynamic{c % 4}"
            g.wait_ge(zsem, 1)
            g.wait_ge(in_sem, 48)
            reg_ctx = g.register("ni_reg")
            ni = reg_ctx.__enter__()
            icol = 0
            waited = set()
            for w in range(n_win):
                if w >= g_bufs:
                    g.wait_ge(mmsem, int(mm_after[w - g_bufs]))
                base = (w % g_bufs) * (smax * d)
                sofs = 0
                for c, p16, n_slots in sched[w]:
                    if c not in waited:
                        g.wait_ge(conv_sems[c], 16)
                        waited.add(c)
                    g.reg_mov(ni, p16)
                    g.dma_gather(
                        out_ap=gbuf[
                            :, base + sofs * d : base + (sofs + n_slots) * d
                        ].rearrange("p (s e) -> p s e", e=d),
                        in_ap=wzb[c * chunk : min((c + 1) * chunk, n_rows), :],
                        idxs_ap=idx_sb[:, icol : icol + p16 // 16],
                        num_idxs=p16,
                        num_idxs_reg=ni,
                        elem_size=d,
                        single_packet=False,
                        queue_num=c % 4,
                    ).then_inc(gsems[c], 16)
                    icol += p16 // 16
                    sofs += n_slots

        @block.vector
        def _(v):
            v.wait_ge(in_sem, 48)
            j = 0
            for w in range(n_win):
                for s in range(slots_per_win[w]):
                    if j >= oh_bufs:
                        v.wait_ge(mmsem, j - oh_bufs + 1)
                    v.tensor_tensor(
                        out=ohbuf[:, (j % oh_bufs) * P : (j % oh_bufs + 1) * P],
                        in0=seg_sb[:, j : j + 1].to_broadcast([P, P]),
                        in1=iota_sb[:],
                        op=mybir.AluOpType.is_equal,
                    ).then_inc(ohsem, 1)
                    j += 1

        @block.tensor
        def _(pe):
            pe.wait_ge(zsem, 1)
            j = 0
            for w in range(n_win):
                base = (w % g_bufs) * (smax * d)
                for c, _, _ in sched[w]:
                    pe.wait_ge(gsems[c], 16 * int(gcount[w][c]))
                if w >= p_bufs:
                    pe.wait_ge(csem, w - p_bufs + 1)
                ns = slots_per_win[w]
                for s in range(ns):
                    pe.wait_ge(ohsem, j + 1)
                    pe.matmul(
                        psums[w % p_bufs][:],
                        lhsT=ohbuf[:, (j % oh_bufs) * P : (j % oh_bufs + 1) * P],
                        rhs=gbuf[:, base + s * d : base + (s + 1) * d],
                        start=(s == 0),
                        stop=(s == ns - 1),
                    ).then_inc(mmsem, 1)
                    j += 1

        @block.scalar
        def _(a):
            for w in range(n_win):
                a.wait_ge(mmsem, int(mm_after[w]))
                if w >= o_bufs:
                    wp = w - o_bufs
                    a.wait_ge(ssems[wp % o_bufs], 16 * (wp // o_bufs + 1))
                a.copy(
                    obuf[:, (w % o_bufs) * d : (w % o_bufs + 1) * d],
                    psums[w % p_bufs][:],
                ).then_inc(csem, 1)

    return nc


def _build_program5(n_rows, d, plan, g_bufs=4, oh_bufs=6, p_bufs=2, o_bufs=4):
    """v5: exact _build_program2 skeleton, but the table is converted once to
    bf16 in DRAM (SWDGE cast-DMA prologue) and the gathers/one-hots/matmuls
    run in bf16 (single-pass PE)."""
    import contextlib

    import concourse.bass as bass
    import concourse.mybir as mybir
    from concourse import library_config

    sched = plan["sched"]
    chunk = plan["chunk"]
    n_win = len(sched)
    n_chunks = max(c for ws in sched for c, _, _ in ws) + 1
    ic = plan["idxbuf"].shape[2]
    sc = plan["segbuf"].shape[2]

    slots_per_win = [sum(ns for _, _, ns in ws) for ws in sched]
    g_per_win = [len(ws) for ws in sched]
    smax = max(slots_per_win)
    mm_after = np.cumsum(slots_per_win)
    lane_after = {}
    lane_tot = [0] * g_bufs
    for w in range(n_win):
        lane_tot[w % g_bufs] += 16 * g_per_win[w]
        lane_after[w] = lane_tot[w % g_bufs]

    nc = bass.Bass(num_swdge_queues=4)
    wz = nc.declare_dram_parameter("wz", [n_rows, d], mybir.dt.float32, isOutput=False)
    idx = nc.declare_dram_parameter("idx", [P, ic], mybir.dt.int16, isOutput=False)
    seg = nc.declare_dram_parameter("seg", [P, sc], mybir.dt.float32, isOutput=False)
    iota = nc.declare_dram_parameter("iota", [P, P], mybir.dt.float32, isOutput=False)
    out = nc.declare_dram_parameter(
        "out", [n_win * P, d], mybir.dt.float32, isOutput=True
    )
    wzb = nc.dram_tensor("wzb", [n_rows, d], mybir.dt.bfloat16)

    with contextlib.ExitStack() as ctx:
        idx_sb = ctx.enter_context(nc.sbuf_tensor([P, ic], mybir.dt.int16))
        seg_sb = ctx.enter_context(nc.sbuf_tensor([P, sc], mybir.dt.float32))
        iota_sb = ctx.enter_context(nc.sbuf_tensor([P, P], mybir.dt.float32))
        gbuf = ctx.enter_context(
            nc.sbuf_tensor([P, g_bufs * smax * d], mybir.dt.bfloat16)
        )
        ohbuf = ctx.enter_context(nc.sbuf_tensor([P, oh_bufs * P], mybir.dt.bfloat16))
        obuf = ctx.enter_context(nc.sbuf_tensor([P, o_bufs * d], mybir.dt.float32))
        psums = [
            ctx.enter_context(nc.psum_tensor(f"ps{i}", [P, d], mybir.dt.float32))
            for i in range(p_bufs)
        ]
        in_sem = ctx.enter_context(nc.semaphore("in_sem"))
        zsem = ctx.enter_context(nc.semaphore("zsem"))
        conv_sem = ctx.enter_context(nc.semaphore("conv_sem"))
        gsems = [ctx.enter_context(nc.semaphore(f"gsem{i}")) for i in range(g_bufs)]
        ohsem = ctx.enter_context(nc.semaphore("ohsem"))
        mmsem = ctx.enter_context(nc.semaphore("mmsem"))
        csem = ctx.enter_context(nc.semaphore("csem"))
        ssems = [ctx.enter_context(nc.semaphore(f"ssem{i}")) for i in range(o_bufs)]
        block = ctx.enter_context(nc.Block())

        @block.sync
        def _(sync):
            sync.dma_start(idx_sb[:], idx[:]).then_inc(in_sem, 16)
            sync.dma_start(seg_sb[:], seg[:]).then_inc(in_sem, 16)
            sync.dma_start(iota_sb[:], iota[:]).then_inc(in_sem, 16)
            for w in range(n_win):
                sync.wait_ge(csem, w + 1)
                sync.dma_start(
                    out[w * P : (w + 1) * P, :],
                    obuf[:, (w % o_bufs) * d : (w % o_bufs + 1) * d],
                ).then_inc(ssems[w % o_bufs], 16)
            for lane in range(o_bufs):
                n_l = len(range(lane, n_win, o_bufs))
                if n_l:
                    sync.wait_ge(ssems[lane], 16 * n_l)

        @block.gpsimd
        def _(g):
            g.load_library(library_config.mlp)
            g.memset(gbuf[:], 0.0).then_inc(zsem, 1)
            for c in range(n_chunks):
                lo, hi = c * chunk, min((c + 1) * chunk, n_rows)
                g.dma_start(wzb[lo:hi, :], wz[lo:hi, :]).then_inc(conv_sem, 16)
            g.wait_ge(zsem, 1)
            g.wait_ge(in_sem, 48)
            g.wait_ge(conv_sem, 16 * n_chunks)
            reg_ctx = g.register("ni_reg")
            ni = reg_ctx.__enter__()
            icol = 0
            for w in range(n_win):
                if w >= g_bufs:
                    g.wait_ge(mmsem, int(mm_after[w - g_bufs]))
                base = (w % g_bufs) * (smax * d)
                sofs = 0
                for c, p16, n_slots in sched[w]:
                    g.reg_mov(ni, p16)
                    g.dma_gather(
                        out_ap=gbuf[
                            :, base + sofs * d : base + (sofs + n_slots) * d
                        ].rearrange("p (s e) -> p s e", e=d),
                        in_ap=wzb[c * chunk : min((c + 1) * chunk, n_rows), :],
                        idxs_ap=idx_sb[:, icol : icol + p16 // 16],
                        num_idxs=p16,
                        num_idxs_reg=ni,
                        elem_size=d,
                        single_packet=False,
                        queue_num=w % g_bufs % 4,
                    ).then_inc(gsems[w % g_bufs], 16)
                    icol += p16 // 16
                    sofs += n_slots

        @block.vector
        def _(v):
            v.wait_ge(in_sem, 48)
            j = 0
            for w in range(n_win):
                for s in range(slots_per_win[w]):
                    if j >= oh_bufs:
                        v.wait_ge(mmsem, j - oh_bufs + 1)
                    v.tensor_tensor(
                        out=ohbuf[:, (j % oh_bufs) * P : (j % oh_bufs + 1) * P],
                        in0=seg_sb[:, j : j + 1].to_broadcast([P, P]),
                        in1=iota_sb[:],
                        op=mybir.AluOpType.is_equal,
                    ).then_inc(ohsem, 1)
                    j += 1

        @block.tensor
        def _(pe):
            pe.wait_ge(zsem, 1)
            j = 0
            for w in range(n_win):
                base = (w % g_bufs) * (smax * d)
                pe.wait_ge(gsems[w % g_bufs], lane_after[w])
                if w >= p_bufs:
                    pe.wait_ge(csem, w - p_bufs + 1)
                ns = slots_per_win[w]
                for s in range(ns):
                    pe.wait_ge(ohsem, j + 1)
                    pe.matmul(
                        psums[w % p_bufs][:],
                        lhsT=ohbuf[:, (j % oh_bufs) * P : (j % oh_bufs + 1) * P],
                        rhs=gbuf[:, base + s * d : base + (s + 1) * d],
                        start=(s == 0),
                        stop=(s == ns - 1),
                    ).then_inc(mmsem, 1)
                    j += 1

        @block.scalar
        def _(a):
            for w in range(n_win):
                a.wait_ge(mmsem, int(mm_after[w]))
                if w >= o_bufs:
                    wp = w - o_bufs
                    a.wait_ge(ssems[wp % o_bufs], 16 * (wp // o_bufs + 1))
                a.copy(
                    obuf[:, (w % o_bufs) * d : (w % o_bufs + 1) * d],
                    psums[w % p_bufs][:],
                ).then_inc(csem, 1)

    return nc


def _run(weights, indices, offsets, trace=False, v2=True, chunk=None, v3=True):
    from concourse import mybir
    from concourse.bass_utils import run_bass_kernel_spmd

    weights = np.ascontiguousarray(np.asarray(weights), dtype=np.float32)
    t, n, d = weights.shape

    if v3:
        try:
            chunk3 = chunk
            if chunk3 is None:
                chunk3 = -(-n // max(1, -(-n // 32767)))
            plan = _plan3(indices, offsets, n, chunk=chunk3, group=8)
            if plan is None:
                raise ValueError("v3 fast path needs fixed-size bags")
            nc = _build_program3(n, d, plan, g_bufs=2, oh_bufs=8, p_bufs=8, o_bufs=4)
            mybir.codegen_inst_isa_subclasses(nc)
            in_maps = [
                {
                    "wz": weights[i],
                    "idx": np.ascontiguousarray(plan["idxbuf"][i]),
                    "seg": np.ascontiguousarray(plan["segbuf"][i]),
                    "iota": plan["iota"],
                }
                for i in range(t)
            ]
        except Exception:
            in_maps = None
        if in_maps is not None:
            res = run_bass_kernel_spmd(nc, in_maps, list(range(t)), trace=trace)
            out = np.stack([res.results[i]["out"] for i in range(t)], axis=0)
            return out, res

    if v2:
        if chunk is None:
            chunk = -(-n // max(1, -(-n // 32767)))  # even chunks, each <= 32767
        plan = _plan2(indices, offsets, n, chunk=chunk)
        nc = _build_program2(n, d, plan)
        mybir.codegen_inst_isa_subclasses(nc)
        in_maps = [
            {
                "wz": weights[i],
                "idx": np.ascontiguousarray(plan["idxbuf"][i]),
                "seg": np.ascontiguousarray(plan["segbuf"][i]),
                "iota": plan["iota"],
            }
            for i in range(t)
        ]
    else:
        idxbufs, lws, col_ofs, sum_l, need_pad = _plan(indices, offsets, n)
        n_win = np.asarray(offsets).shape[1] // P
        if need_pad:
            wz = np.concatenate([weights, np.zeros((t, 1, d), np.float32)], axis=1)
        else:
            wz = weights
        nc = _build_program(wz.shape[1], d, n_win, lws, col_ofs, sum_l)
        in_maps = [
            {"wz": wz[i], "idx": np.ascontiguousarray(idxbufs[i])} for i in range(t)
        ]
    res = run_bass_kernel_spmd(nc, in_maps, list(range(t)), trace=trace)
    out = np.stack([res.results[i]["out"] for i in range(t)], axis=0)
    return out, res


def kernel(weights, indices, offsets):
    out, _ = _run(weights, indices, offsets)
    return out



# revision 18
# speedup vs baseline: 1.1175x; 1.0543x over previous
"""MergedEmbeddingBag forward (sum pooling) on 8 Trainium2 NeuronCores.

Strategy (table-parallel, per sharding hint): core t owns table t, SPMD.

Default (v3) path, fixed-size bags: the f32 table is converted once on
device to a bf16 DRAM copy (SWDGE cast-DMA, overlapped with the pipeline
start).  Gathers then run as one big dma_gather per (4-window group, table
chunk) - chunks of <=25000 rows keep the indices within the int16 HW limit -
writing [occurrence_partition, slot, 128 bf16] tiles.  Pooling is a one-hot
matmul per 128-occurrence slot: DVE builds the one-hot (seg==iota, bf16 out)
from host-precomputed segment labels, PE accumulates single-pass bf16
matmuls into a per-window PSUM tile, ACT copies it out, sync DMAs it back.
All indices/segments are precomputed on the host and streamed in as data.

Measured bottleneck (from NTFF traces): the SWDGE gather pipeline sustains
~2.9 ns per descriptor aggregate across the 4 queue contexts regardless of
descriptor size (256B vs 512B) or call size, so runtime ~= #descriptors x
2.9ns; bf16 halves bytes (engine relief) but not descriptors.
single_packet=True crashes the device; dynamic_dma_scratch_size has no
effect on the cadence.

Fallbacks: general offsets use the v2 f32 path (same one-hot pooling,
per-(window, chunk) gathers); v1 (indirect DMA + DVE strided reduce) kept
for reference.
"""

import sys

sys.path.insert(0, "/opt/trn_rl_repo")

import numpy as np

# Problem geometry (hardcoded per contract; the builder itself is generic).
T = 8
N = 100000
D = 128
B = 16384
TOTAL = 327680
P = 128  # partitions / bags per window
W = B // P  # 128 windows


def _build_program(n_rows, d, n_win, lws, col_ofs, sum_l, g_bufs=6, o_bufs=4):
    """Build the SPMD raw-Bass program (explicit semaphores).

    Pipeline: gpsimd issues SWDGE indirect gathers (bag-major into SBUF),
    DVE does one strided reduce per window, SP (sync) stores pooled tiles.

    n_rows: rows in the (possibly zero-row-extended) weight table
    lws[w]: items per bag in window w (uniform within a window, padded)
    col_ofs[w]: column offset of window w's index block in the idx buffer
    sum_l: total index columns (sum of lws)
    """
    import concourse.bass as bass
    import concourse.mybir as mybir

    lmax = max(lws)
    nc = bass.Bass(num_swdge_queues=4)
    wz = nc.declare_dram_parameter("wz", [n_rows, d], mybir.dt.float32, isOutput=False)
    idx = nc.declare_dram_parameter("idx", [P, sum_l], mybir.dt.int32, isOutput=False)
    out = nc.declare_dram_parameter(
        "out", [n_win * P, d], mybir.dt.float32, isOutput=True
    )

    import contextlib

    with contextlib.ExitStack() as ctx:
        idx_sb = ctx.enter_context(nc.sbuf_tensor([P, sum_l], mybir.dt.int32))
        gbuf = ctx.enter_context(
            nc.sbuf_tensor([P, g_bufs * lmax * d], mybir.dt.float32)
        )
        obuf = ctx.enter_context(nc.sbuf_tensor([P, o_bufs * d], mybir.dt.float32))
        idx_sem = ctx.enter_context(nc.semaphore("idx_sem"))
        # One completion sem per buffer slot: at most one DMA in flight per
        # sem, so ge-16k waits are race-free.
        gsems = [ctx.enter_context(nc.semaphore(f"gsem{i}")) for i in range(g_bufs)]
        ssems = [ctx.enter_context(nc.semaphore(f"ssem{i}")) for i in range(o_bufs)]
        rsem = ctx.enter_context(nc.semaphore("rsem"))
        block = ctx.enter_context(nc.Block())

        def gslot(w):
            s = w % g_bufs
            return gbuf[:, s * lmax * d : s * lmax * d + lws[w] * d]

        def oslot(w):
            s = w % o_bufs
            return obuf[:, s * d : (s + 1) * d]

        @block.sync
        def _(sync):
            sync.dma_start(idx_sb[:], idx[:]).then_inc(idx_sem, 16)
            for w in range(n_win):
                sync.wait_ge(rsem, w + 1)
                sync.dma_start(out[w * P : (w + 1) * P, :], oslot(w)).then_inc(
                    ssems[w % o_bufs], 16
                )
            for lane in range(o_bufs):
                n_l = len(range(lane, n_win, o_bufs))
                if n_l:
                    sync.wait_ge(ssems[lane], 16 * n_l)

        # HW indirect DMA supports exactly one offset per partition per
        # instruction ([P,1] offsets -> [P,elem] dest), so a window of L
        # items takes L gather instructions.  All of window w's gathers
        # inc the window's lane sem; the consumer waits for the lane's
        # cumulative total, which is race-free because the next window on
        # a lane only starts after that wait was consumed (via rsem).
        lane_after = {}
        lane_tot = [0] * g_bufs
        for w in range(n_win):
            lane_tot[w % g_bufs] += 16 * lws[w]
            lane_after[w] = lane_tot[w % g_bufs]

        @block.gpsimd
        def _(g):
            g.wait_ge(idx_sem, 16)
            for w in range(n_win):
                if w >= g_bufs:
                    g.wait_ge(rsem, w - g_bufs + 1)
                base = (w % g_bufs) * (lmax * d)
                for l in range(lws[w]):
                    inst = g.indirect_dma_start(
                        out=gbuf[:, base + l * d : base + (l + 1) * d],
                        out_offset=None,
                        in_=wz[:],
                        in_offset=bass.IndirectOffsetOnAxis(
                            ap=idx_sb[:, col_ofs[w] + l : col_ofs[w] + l + 1],
                            axis=0,
                        ),
                    ).then_inc(gsems[w % g_bufs], 16)
                    # Spread SWDGE desc-gen across all 4 queue contexts —
                    # measured 3.6x throughput vs the single default queue.
                    q = (w * lws[w] + l) % 4
                    if q:
                        inst.ins.queue = f"qPoolDynamic{q}"

        @block.vector
        def _(v):
            for w in range(n_win):
                v.wait_ge(gsems[w % g_bufs], lane_after[w])
                if w >= o_bufs:
                    wp = w - o_bufs
                    v.wait_ge(ssems[wp % o_bufs], 16 * (wp // o_bufs + 1))
                v.reduce_sum(
                    oslot(w),
                    gslot(w).rearrange("p (l e) -> p e l", e=d),
                    axis=mybir.AxisListType.X,
                ).then_inc(rsem, 1)

    return nc


def _plan(indices, offsets, pad_row):
    """Host-side planning: per-table padded bag-major index buffers.

    pad_row: index of the appended all-zeros row (= original table row count).
    Returns (idxbufs [T, P, sum_l] int32, lws, col_ofs, sum_l, need_pad).
    """
    idx64 = np.ascontiguousarray(indices).astype(np.int64)
    off = np.ascontiguousarray(offsets).astype(np.int64)
    t, total = idx64.shape
    b = off.shape[1]
    n_win = b // P

    ends = np.concatenate([off[:, 1:], np.full((t, 1), total, np.int64)], axis=1)
    lens = np.clip(ends - off, 0, None)  # [T, B]

    l_uniform = total // b
    fixed = (
        total == b * l_uniform
        and (lens == l_uniform).all()
        and (off == np.arange(b, dtype=np.int64) * l_uniform).all()
    )

    if fixed:
        lws = [l_uniform] * n_win
        col_ofs = [w * l_uniform for w in range(n_win)]
        sum_l = n_win * l_uniform
        # [t, b, l] -> [t, p, w*L+l]
        bufs = (
            idx64.reshape(t, n_win, P, l_uniform)
            .transpose(0, 2, 1, 3)
            .reshape(t, P, sum_l)
            .astype(np.int32)
        )
        return bufs, lws, col_ofs, sum_l, False

    lws = []
    col_ofs = []
    blocks = []
    need_pad = False
    for w in range(n_win):
        b0 = w * P
        lens_w = lens[:, b0 : b0 + P]  # [T, P]
        lw = max(1, int(lens_w.max()))
        if (lens_w != lw).any():
            need_pad = True
        l_grid = np.arange(lw, dtype=np.int64)
        pos = off[:, b0 : b0 + P, None] + l_grid[None, None, :]  # [T, P, lw]
        valid = l_grid[None, None, :] < lens_w[:, :, None]
        gathered = np.take_along_axis(
            idx64, pos.clip(0, total - 1).reshape(t, -1), axis=1
        ).reshape(t, P, lw)
        blocks.append(np.where(valid, gathered, pad_row).astype(np.int32))
        col_ofs.append(sum(lws))
        lws.append(lw)
    sum_l = sum(lws)
    bufs = np.concatenate(blocks, axis=2)
    return bufs, lws, col_ofs, sum_l, need_pad


def _plan2(indices, offsets, n_rows, chunk=25000):
    """Host planning for the dma_gather path.

    Rows of each 128-bag window are stable-sorted by table chunk
    (idx // chunk) so each run's local indices fit int16.  Runs are padded
    to a multiple of 16 (shared across tables) with dummy index 0; dummy /
    stale positions carry seg = -1 so the one-hot pooling drops them.

    Returns dict with per-table device buffers and the static schedule.
    """
    idx64 = np.ascontiguousarray(indices).astype(np.int64)
    off = np.ascontiguousarray(offsets).astype(np.int64)
    t, total = idx64.shape
    b = off.shape[1]
    n_win = b // P
    n_chunks = -(-n_rows // chunk)
    assert chunk <= 32767

    ends = np.concatenate([off[:, 1:], np.full((t, 1), total, np.int64)], axis=1)
    lens = np.clip(ends - off, 0, None)  # [T, B]

    # Per window, per table: positions and their bag (seg) in window-local
    # terms, sorted by chunk.
    idx_cols = []   # per-(w,c) int16 [T, P16] local indices
    seg_cols = []   # per-(w,slot) f32 [T, 128] segs
    sched = []      # per window: list of (chunk_id, P16, n_slots)
    l_uni = total // b
    uniform = (
        total == b * l_uni
        and (lens == l_uni).all()
        and (off == np.arange(b, dtype=np.int64) * l_uni).all()
    )
    seg_uni = np.repeat(np.arange(P), l_uni)

    for w in range(n_win):
        b0 = w * P
        per_table = []  # (idx_sorted, seg_sorted, chunk_sorted) per table
        for i in range(t):
            if uniform:
                ix = idx64[i, b0 * l_uni : (b0 + P) * l_uni]
                segs = seg_uni
            else:
                ls = lens[i, b0 : b0 + P]
                segs = np.repeat(np.arange(P), ls)
                pos = np.concatenate(
                    [
                        np.arange(off[i, b0 + j], off[i, b0 + j] + ls[j])
                        for j in range(P)
                    ]
                ) if ls.sum() else np.zeros(0, np.int64)
                ix = idx64[i, pos] if len(pos) else np.zeros(0, np.int64)
            c = ix // chunk
            order = np.argsort(c, kind="stable")
            per_table.append((ix[order], segs[order], c[order]))
        wsched = []
        for c in range(n_chunks):
            ns = [int((pt[2] == c).sum()) for pt in per_table]
            mx = max(ns)
            if mx == 0:
                continue
            p16 = -(-mx // 16) * 16
            n_slots = -(-p16 // P)
            ib = np.zeros((t, p16), np.int16)
            sb = np.full((t, n_slots * P), -1.0, np.float32)
            for i in range(t):
                sel = per_table[i][2] == c
                k = ns[i]
                ib[i, :k] = (per_table[i][0][sel] - c * chunk).astype(np.int16)
                sb[i, :k] = per_table[i][1][sel].astype(np.float32)
            idx_cols.append(ib)
            seg_cols.append(sb)
            wsched.append((c, p16, n_slots))
        if not wsched:
            # Empty window: one dummy run so the psum still gets written
            # (with zeros) before the copy-out.
            idx_cols.append(np.zeros((t, 16), np.int16))
            seg_cols.append(np.full((t, P), -1.0, np.float32))
            wsched.append((0, 16, 1))
        sched.append(wsched)

    # Device idx buffer: wrapped [16, cols] replicated to 128 partitions.
    iparts = []
    for ib in idx_cols:
        t_, p16 = ib.shape
        iparts.append(ib.reshape(t_, p16 // 16, 16).transpose(0, 2, 1))
    idxbuf16 = np.concatenate(iparts, axis=2)  # [T, 16, IC]
    idxbuf = np.tile(idxbuf16, (1, 8, 1))  # [T, 128, IC]
    # Device seg buffer: [T, 128, n_slots_total] (seg of (partition, slot)).
    sparts = [sb.reshape(t, -1, P).transpose(0, 2, 1) for sb in seg_cols]
    segbuf = np.concatenate(sparts, axis=2)
    iota = np.tile(np.arange(P, dtype=np.float32)[None, :], (P, 1))
    return {
        "sched": sched,
        "idxbuf": np.ascontiguousarray(idxbuf),
        "segbuf": np.ascontiguousarray(segbuf),
        "iota": iota,
        "chunk": chunk,
    }


def _build_program2(n_rows, d, plan, g_bufs=4, oh_bufs=6, p_bufs=2, o_bufs=4):
    """dma_gather + one-hot-matmul pooling program (raw Bass)."""
    import contextlib

    import concourse.bass as bass
    import concourse.mybir as mybir
    from concourse import library_config

    sched = plan["sched"]
    chunk = plan["chunk"]
    n_win = len(sched)
    ic = plan["idxbuf"].shape[2]
    sc = plan["segbuf"].shape[2]

    # Static per-window derived counts.
    slots_per_win = [sum(ns for _, _, ns in ws) for ws in sched]
    g_per_win = [len(ws) for ws in sched]
    smax = max(slots_per_win)
    mm_after = np.cumsum(slots_per_win)  # matmuls (= slots) completed after w
    lane_after = {}
    lane_tot = [0] * g_bufs
    for w in range(n_win):
        lane_tot[w % g_bufs] += 16 * g_per_win[w]
        lane_after[w] = lane_tot[w % g_bufs]

    nc = bass.Bass(num_swdge_queues=4)
    wz = nc.declare_dram_parameter("wz", [n_rows, d], mybir.dt.float32, isOutput=False)
    idx = nc.declare_dram_parameter("idx", [P, ic], mybir.dt.int16, isOutput=False)
    seg = nc.declare_dram_parameter("seg", [P, sc], mybir.dt.float32, isOutput=False)
    iota = nc.declare_dram_parameter("iota", [P, P], mybir.dt.float32, isOutput=False)
    out = nc.declare_dram_parameter(
        "out", [n_win * P, d], mybir.dt.float32, isOutput=True
    )

    with contextlib.ExitStack() as ctx:
        idx_sb = ctx.enter_context(nc.sbuf_tensor([P, ic], mybir.dt.int16))
        seg_sb = ctx.enter_context(nc.sbuf_tensor([P, sc], mybir.dt.float32))
        iota_sb = ctx.enter_context(nc.sbuf_tensor([P, P], mybir.dt.float32))
        gbuf = ctx.enter_context(
            nc.sbuf_tensor([P, g_bufs * smax * d], mybir.dt.float32)
        )
        ohbuf = ctx.enter_context(nc.sbuf_tensor([P, oh_bufs * P], mybir.dt.float32))
        obuf = ctx.enter_context(nc.sbuf_tensor([P, o_bufs * d], mybir.dt.float32))
        psums = [
            ctx.enter_context(nc.psum_tensor(f"ps{i}", [P, d], mybir.dt.float32))
            for i in range(p_bufs)
        ]
        in_sem = ctx.enter_context(nc.semaphore("in_sem"))
        zsem = ctx.enter_context(nc.semaphore("zsem"))
        gsems = [ctx.enter_context(nc.semaphore(f"gsem{i}")) for i in range(g_bufs)]
        ohsem = ctx.enter_context(nc.semaphore("ohsem"))
        mmsem = ctx.enter_context(nc.semaphore("mmsem"))
        csem = ctx.enter_context(nc.semaphore("csem"))
        ssems = [ctx.enter_context(nc.semaphore(f"ssem{i}")) for i in range(o_bufs)]
        block = ctx.enter_context(nc.Block())

        @block.sync
        def _(sync):
            sync.dma_start(idx_sb[:], idx[:]).then_inc(in_sem, 16)
            sync.dma_start(seg_sb[:], seg[:]).then_inc(in_sem, 16)
            sync.dma_start(iota_sb[:], iota[:]).then_inc(in_sem, 16)
            for w in range(n_win):
                sync.wait_ge(csem, w + 1)
                sync.dma_start(
                    out[w * P : (w + 1) * P, :],
                    obuf[:, (w % o_bufs) * d : (w % o_bufs + 1) * d],
                ).then_inc(ssems[w % o_bufs], 16)
            for lane in range(o_bufs):
                n_l = len(range(lane, n_win, o_bufs))
                if n_l:
                    sync.wait_ge(ssems[lane], 16 * n_l)

        @block.gpsimd
        def _(g):
            g.load_library(library_config.mlp)
            # First-ever use of gbuf: ensure finite contents so one-hot
            # zero-columns can't turn stale NaNs into NaN outputs.
            g.memset(gbuf[:], 0.0).then_inc(zsem, 1)
            g.wait_ge(zsem, 1)
            g.wait_ge(in_sem, 48)
            reg_ctx = g.register("ni_reg")
            ni = reg_ctx.__enter__()
            icol = 0
            for w in range(n_win):
                if w >= g_bufs:
                    g.wait_ge(mmsem, int(mm_after[w - g_bufs]))
                base = (w % g_bufs) * (smax * d)
                sofs = 0
                for c, p16, n_slots in sched[w]:
                    g.reg_mov(ni, p16)
                    g.dma_gather(
                        out_ap=gbuf[
                            :, base + sofs * d : base + (sofs + n_slots) * d
                        ].rearrange("p (s e) -> p s e", e=d),
                        in_ap=wz[c * chunk : min((c + 1) * chunk, n_rows), :],
                        idxs_ap=idx_sb[:, icol : icol + p16 // 16],
                        num_idxs=p16,
                        num_idxs_reg=ni,
                        elem_size=d,
                        single_packet=False,
                        queue_num=w % g_bufs % 4,
                    ).then_inc(gsems[w % g_bufs], 16)
                    icol += p16 // 16
                    sofs += n_slots

        @block.vector
        def _(v):
            v.wait_ge(in_sem, 48)
            j = 0  # global slot index
            for w in range(n_win):
                for s in range(slots_per_win[w]):
                    if j >= oh_bufs:
                        v.wait_ge(mmsem, j - oh_bufs + 1)
                    v.tensor_tensor(
                        out=ohbuf[:, (j % oh_bufs) * P : (j % oh_bufs + 1) * P],
                        in0=seg_sb[:, j : j + 1].to_broadcast([P, P]),
                        in1=iota_sb[:],
                        op=mybir.AluOpType.is_equal,
                    ).then_inc(ohsem, 1)
                    j += 1

        @block.tensor
        def _(pe):
            pe.wait_ge(zsem, 1)
            j = 0
            for w in range(n_win):
                base = (w % g_bufs) * (smax * d)
                pe.wait_ge(gsems[w % g_bufs], lane_after[w])
                if w >= p_bufs:
                    pe.wait_ge(csem, w - p_bufs + 1)
                ns = slots_per_win[w]
                for s in range(ns):
                    pe.wait_ge(ohsem, j + 1)
                    pe.matmul(
                        psums[w % p_bufs][:],
                        lhsT=ohbuf[:, (j % oh_bufs) * P : (j % oh_bufs + 1) * P],
                        rhs=gbuf[:, base + s * d : base + (s + 1) * d],
                        start=(s == 0),
                        stop=(s == ns - 1),
                    ).then_inc(mmsem, 1)
                    j += 1

        @block.scalar
        def _(a):
            for w in range(n_win):
                a.wait_ge(mmsem, int(mm_after[w]))
                if w >= o_bufs:
                    wp = w - o_bufs
                    a.wait_ge(ssems[wp % o_bufs], 16 * (wp // o_bufs + 1))
                a.copy(
                    obuf[:, (w % o_bufs) * d : (w % o_bufs + 1) * d],
                    psums[w % p_bufs][:],
                ).then_inc(csem, 1)

    return nc


def _plan3(indices, offsets, n_rows, chunk=25000, group=4):
    """Host planning for the v3 (bf16, merged-call) path.

    Fixed-size-bag fast path only (falls back to v2 otherwise).  Windows of
    128 bags; groups of `group` windows; per (group, chunk) one dma_gather
    whose index stream is the concatenation of the group's windows'
    chunk-local occurrences, each window's section padded to a multiple of
    128 (pad idx 0 with seg -1, dropped by the one-hot).
    """
    idx64 = np.ascontiguousarray(indices).astype(np.int64)
    off = np.ascontiguousarray(offsets).astype(np.int64)
    t, total = idx64.shape
    b = off.shape[1]
    n_win = b // P
    n_chunks = -(-n_rows // chunk)
    l_uni = total // b
    if not (
        total == b * l_uni
        and (off == np.arange(b, dtype=np.int64) * l_uni).all()
        and n_win % group == 0
    ):
        return None
    n_grp = n_win // group

    # idx stream order = (g, c, w) [one gather per (g, c)]; seg (one-hot)
    # order = (g, w, c) = tensor/vector consumption order.
    idx_cols = []  # per (g, c): int16 [T, n16cols*16]
    seg_blocks = {}  # (g, c, wl) -> float32 [T, a128]
    sched = []  # per g: list of (c, num_idxs, n_slots, [per-w n_slots])
    bags = np.repeat(np.arange(P), l_uni)  # [P*l_uni] bag of each pos in window

    iw = idx64.reshape(t, n_win, P * l_uni)
    cw = iw // chunk  # chunk of each occurrence
    lw = iw - cw * chunk  # chunk-local index

    for g in range(n_grp):
        gsched = []
        for c in range(n_chunks):
            ibs, wslots = [], []
            for wl in range(group):
                w = g * group + wl
                sel = cw[:, w, :] == c  # [T, P*l]
                cnt = sel.sum(axis=1)  # [T]
                mx = int(cnt.max())
                a128 = max(128, -(-mx // 128) * 128)
                ib = np.zeros((t, a128), np.int16)
                sb = np.full((t, a128), -1.0, np.float32)
                for i in range(t):
                    k = int(cnt[i])
                    ib[i, :k] = lw[i, w, sel[i]].astype(np.int16)
                    sb[i, :k] = bags[sel[i]].astype(np.float32)
                ibs.append(ib)
                seg_blocks[(g, c, wl)] = sb
                wslots.append(a128 // 128)
            ib = np.concatenate(ibs, axis=1)
            num = ib.shape[1]
            gsched.append((c, num, num // 128, wslots))
            idx_cols.append(ib)
        sched.append(gsched)
    # seg columns in consumption order: (g, wl, c)
    seg_cols = [
        seg_blocks[(g, c, wl)]
        for g in range(n_grp)
        for wl in range(group)
        for c in range(n_chunks)
    ]

    # wrap idx int16 into [16, cols] replicated to 128 partitions
    iparts = []
    for ib in idx_cols:
        t_, n = ib.shape
        iparts.append(ib.reshape(t_, n // 16, 16).transpose(0, 2, 1))
    idxbuf = np.tile(np.concatenate(iparts, axis=2), (1, 8, 1))  # [T,128,IC]
    sparts = [sb.reshape(t, -1, P).transpose(0, 2, 1) for sb in seg_cols]
    segbuf = np.concatenate(sparts, axis=2)  # [T, 128, n_slots_tot]
    iota = np.tile(np.arange(P, dtype=np.float32)[None, :], (P, 1))
    return {
        "sched": sched,
        "idxbuf": np.ascontiguousarray(idxbuf),
        "segbuf": np.ascontiguousarray(segbuf),
        "iota": iota,
        "chunk": chunk,
        "group": group,
        "n_chunks": n_chunks,
    }


def _plan8(indices, offsets, n_rows, chunk=25000, group=8):
    """v8: like _plan3 but per-(group, chunk) calls pad only to 16 indices
    (stale-tail slots handle the rest), and slots may straddle window
    boundaries; straddling slots get one matmul per window with segs masked
    to that window (-1 elsewhere).

    Returns plan with sched per group: list over chunks of
    (c, num_idxs16, n_slots, parts) where parts = list over matmuls of
    (slot, window_local, is_first_for_window, is_last_for_window) resolved
    later; here we return per-part (slot, wl) and per-window first/last
    bookkeeping is done by the builder via win_parts.
    """
    idx64 = np.ascontiguousarray(indices).astype(np.int64)
    off = np.ascontiguousarray(offsets).astype(np.int64)
    t, total = idx64.shape
    b = off.shape[1]
    n_win = b // P
    n_chunks = -(-n_rows // chunk)
    l_uni = total // b
    if not (
        total == b * l_uni
        and (off == np.arange(b, dtype=np.int64) * l_uni).all()
        and n_win % group == 0
    ):
        return None
    n_grp = n_win // group

    bags = np.repeat(np.arange(P), l_uni)
    iw = idx64.reshape(t, n_win, P * l_uni)
    cw = iw // chunk
    lw = iw - cw * chunk

    idx_cols = []  # per (g,c): int16 [T, ceil16]
    seg_cols = []  # per matmul part: f32 [T, 128]
    sched = []  # per g: list of (c, num16, n_slots, parts[(slot, wl)])
    for g in range(n_grp):
        gsched = []
        for c in range(n_chunks):
            sels = []
            cnts = np.zeros((group, t), np.int64)
            for wl in range(group):
                w = g * group + wl
                sel = cw[:, w, :] == c
                sels.append(sel)
                cnts[wl] = sel.sum(axis=1)
            # per-table window section boundaries (padded to the max so the
            # stream layout is shared across tables SPMD)
            secl = cnts.max(axis=1)  # [group] shared section lengths
            starts = np.concatenate([[0], np.cumsum(secl)])
            tot_cols = int(starts[-1])
            num16 = max(16, -(-tot_cols // 16) * 16)
            n_slots = max(1, -(-tot_cols // 128))
            ib = np.zeros((t, num16), np.int16)
            sg = np.full((t, group, n_slots * 128), -1.0, np.float32)
            for i in range(t):
                for wl in range(group):
                    s0 = int(starts[wl])
                    k = int(cnts[wl][i])
                    ib[i, s0 : s0 + k] = lw[i, g * group + wl, sels[wl][i]].astype(
                        np.int16
                    )
                    sg[i, wl, s0 : s0 + k] = bags[sels[wl][i]].astype(np.float32)
            # parts: for each slot, which windows have any live col
            parts = []
            for s in range(n_slots):
                for wl in range(group):
                    lo, hi = int(starts[wl]), int(starts[wl + 1])
                    if lo < (s + 1) * 128 and hi > s * 128:
                        parts.append((s, wl))
                        seg_cols.append(
                            np.ascontiguousarray(sg[:, wl, s * 128 : (s + 1) * 128])
                        )
            gsched.append((c, num16, n_slots, parts))
            idx_cols.append(ib)
        sched.append(gsched)

    iparts = []
    for ib in idx_cols:
        t_, n = ib.shape
        iparts.append(ib.reshape(t_, n // 16, 16).transpose(0, 2, 1))
    idxbuf = np.tile(np.concatenate(iparts, axis=2), (1, 8, 1))
    sparts = [sb.reshape(t, 1, P).transpose(0, 2, 1) for sb in seg_cols]
    segbuf = np.concatenate(sparts, axis=2)
    iota = np.tile(np.arange(P, dtype=np.float32)[None, :], (P, 1))
    return {
        "sched": sched,
        "idxbuf": np.ascontiguousarray(idxbuf),
        "segbuf": np.ascontiguousarray(segbuf),
        "iota": iota,
        "chunk": chunk,
        "group": group,
        "n_chunks": n_chunks,
    }


def _build_program8(n_rows, d, plan, g_bufs=2, oh_bufs=8, p_bufs=8, o_bufs=4):
    """v8 builder: merged 16-padded gathers; boundary slots matmul'd once per
    live window with window-masked segs.  Matmul/one-hot order = stream
    order (g, c, slot, window-part); psum start/stop per window derived from
    each window's global first/last part."""
    import contextlib

    import concourse.bass as bass
    import concourse.mybir as mybir
    from concourse import library_config

    sched = plan["sched"]
    chunk = plan["chunk"]
    group = plan["group"]
    n_chunks = plan["n_chunks"]
    n_grp = len(sched)
    n_win = n_grp * group
    ic = plan["idxbuf"].shape[2]
    sc = plan["segbuf"].shape[2]

    # global matmul (part) list in stream order, with per-window first/last
    all_parts = []  # (g, c_i, slot, wl)
    for g in range(n_grp):
        for c_i, (c, num16, n_slots, parts) in enumerate(sched[g]):
            for (s, wl) in parts:
                all_parts.append((g, c_i, s, wl))
    n_mm = len(all_parts)
    win_first = {}
    win_last = {}
    for j, (g, c_i, s, wl) in enumerate(all_parts):
        wg = g * group + wl
        if wg not in win_first:
            win_first[wg] = j
        win_last[wg] = j
    # matmuls completed after window wg's last part
    mm_after_win = [win_last[w] + 1 for w in range(n_win)]
    grp_slots = [sum(ns for _, _, ns, _ in sched[g]) for g in range(n_grp)]
    smax = max(grp_slots)
    grp_parts = [sum(len(p) for _, _, _, p in sched[g]) for g in range(n_grp)]
    mm_after_grp = np.cumsum(grp_parts)

    nc = bass.Bass(num_swdge_queues=4)
    wz = nc.declare_dram_parameter("wz", [n_rows, d], mybir.dt.float32, isOutput=False)
    idx = nc.declare_dram_parameter("idx", [P, ic], mybir.dt.int16, isOutput=False)
    seg = nc.declare_dram_parameter("seg", [P, sc], mybir.dt.float32, isOutput=False)
    iota = nc.declare_dram_parameter("iota", [P, P], mybir.dt.float32, isOutput=False)
    out = nc.declare_dram_parameter(
        "out", [n_win * P, d], mybir.dt.float32, isOutput=True
    )
    wzb = nc.dram_tensor("wzb", [n_rows, d], mybir.dt.bfloat16)

    with contextlib.ExitStack() as ctx:
        idx_sb = ctx.enter_context(nc.sbuf_tensor([P, ic], mybir.dt.int16))
        seg_sb = ctx.enter_context(nc.sbuf_tensor([P, sc], mybir.dt.float32))
        iota_sb = ctx.enter_context(nc.sbuf_tensor([P, P], mybir.dt.float32))
        gbuf = ctx.enter_context(
            nc.sbuf_tensor([P, g_bufs * smax * d], mybir.dt.bfloat16)
        )
        ohbuf = ctx.enter_context(nc.sbuf_tensor([P, oh_bufs * P], mybir.dt.bfloat16))
        obuf = ctx.enter_context(nc.sbuf_tensor([P, o_bufs * d], mybir.dt.float32))
        psums = [
            ctx.enter_context(nc.psum_tensor(f"ps{i}", [P, d], mybir.dt.float32))
            for i in range(p_bufs)
        ]
        in_sem = ctx.enter_context(nc.semaphore("in_sem"))
        zsem = ctx.enter_context(nc.semaphore("zsem"))
        conv_sems = [
            ctx.enter_context(nc.semaphore(f"conv{c}")) for c in range(n_chunks)
        ]
        gsems = [ctx.enter_context(nc.semaphore(f"gsem{c}")) for c in range(n_chunks)]
        ohsem = ctx.enter_context(nc.semaphore("ohsem"))
        mmsem = ctx.enter_context(nc.semaphore("mmsem"))
        csem = ctx.enter_context(nc.semaphore("csem"))
        ssems = [ctx.enter_context(nc.semaphore(f"ssem{i}")) for i in range(o_bufs)]
        block = ctx.enter_context(nc.Block())

        @block.sync
        def _(sync):
            sync.dma_start(idx_sb[:], idx[:]).then_inc(in_sem, 16)
            sync.dma_start(seg_sb[:], seg[:]).then_inc(in_sem, 16)
            sync.dma_start(iota_sb[:], iota[:]).then_inc(in_sem, 16)
            for w in range(n_win):
                sync.wait_ge(csem, w + 1)
                sync.dma_start(
                    out[w * P : (w + 1) * P, :],
                    obuf[:, (w % o_bufs) * d : (w % o_bufs + 1) * d],
                ).then_inc(ssems[w % o_bufs], 16)
            for lane in range(o_bufs):
                n_l = len(range(lane, n_win, o_bufs))
                if n_l:
                    sync.wait_ge(ssems[lane], 16 * n_l)

        @block.gpsimd
        def _(g):
            g.load_library(library_config.mlp)
            g.memset(gbuf[:], 0.0).then_inc(zsem, 1)
            for c in range(n_chunks):
                lo, hi = c * chunk, min((c + 1) * chunk, n_rows)
                inst = g.dma_start(wzb[lo:hi, :], wz[lo:hi, :]).then_inc(
                    conv_sems[c], 16
                )
                if c % 4:
                    inst.ins.queue = f"qPoolDynamic{c % 4}"
            g.wait_ge(zsem, 1)
            g.wait_ge(in_sem, 48)
            reg_ctx = g.register("ni_reg")
            ni = reg_ctx.__enter__()
            icol = 0
            waited = set()
            for gi in range(n_grp):
                if gi >= g_bufs:
                    g.wait_ge(mmsem, int(mm_after_grp[gi - g_bufs]))
                base = (gi % g_bufs) * (smax * d)
                sofs = 0
                for c, num16, n_slots, parts in sched[gi]:
                    if c not in waited:
                        g.wait_ge(conv_sems[c], 16)
                        waited.add(c)
                    g.reg_mov(ni, num16)
                    g.dma_gather(
                        out_ap=gbuf[
                            :, base + sofs * d : base + (sofs + n_slots) * d
                        ].rearrange("p (s e) -> p s e", e=d),
                        in_ap=wzb[c * chunk : min((c + 1) * chunk, n_rows), :],
                        idxs_ap=idx_sb[:, icol : icol + num16 // 16],
                        num_idxs=num16,
                        num_idxs_reg=ni,
                        elem_size=d,
                        single_packet=False,
                        queue_num=c % 4,
                    ).then_inc(gsems[c], 16)
                    icol += num16 // 16
                    sofs += n_slots

        @block.vector
        def _(v):
            v.wait_ge(in_sem, 48)
            for j in range(n_mm):
                if j >= oh_bufs:
                    v.wait_ge(mmsem, j - oh_bufs + 1)
                v.tensor_tensor(
                    out=ohbuf[:, (j % oh_bufs) * P : (j % oh_bufs + 1) * P],
                    in0=seg_sb[:, j : j + 1].to_broadcast([P, P]),
                    in1=iota_sb[:],
                    op=mybir.AluOpType.is_equal,
                ).then_inc(ohsem, 1)

        @block.tensor
        def _(pe):
            pe.wait_ge(zsem, 1)
            j = 0
            for gi in range(n_grp):
                base = (gi % g_bufs) * (smax * d)
                # slot offset of each chunk-call's region within the group
                call_sofs = []
                so = 0
                for c, num16, n_slots, parts in sched[gi]:
                    call_sofs.append(so)
                    so += n_slots
                for c_i, (c, num16, n_slots, parts) in enumerate(sched[gi]):
                    pe.wait_ge(gsems[c], 16 * (gi + 1))
                    for (s, wl) in parts:
                        wg = gi * group + wl
                        if win_first[wg] == j and wg >= p_bufs:
                            pe.wait_ge(csem, wg - p_bufs + 1)
                        slot = call_sofs[c_i] + s
                        pe.wait_ge(ohsem, j + 1)
                        pe.matmul(
                            psums[wg % p_bufs][:],
                            lhsT=ohbuf[:, (j % oh_bufs) * P : (j % oh_bufs + 1) * P],
                            rhs=gbuf[:, base + slot * d : base + (slot + 1) * d],
                            start=(win_first[wg] == j),
                            stop=(win_last[wg] == j),
                            skip_group_check=True,
                        ).then_inc(mmsem, 1)
                        j += 1

        @block.scalar
        def _(a):
            for w in range(n_win):
                a.wait_ge(mmsem, int(mm_after_win[w]))
                if w >= o_bufs:
                    wp = w - o_bufs
                    a.wait_ge(ssems[wp % o_bufs], 16 * (wp // o_bufs + 1))
                a.copy(
                    obuf[:, (w % o_bufs) * d : (w % o_bufs + 1) * d],
                    psums[w % p_bufs][:],
                ).then_inc(csem, 1)

    return nc


def _build_program3(
    n_rows, d, plan, g_bufs=2, oh_bufs=8, p_bufs=8, o_bufs=4, single_packet=False
):
    """bf16 convert + merged dma_gather + one-hot matmul pooling (raw Bass)."""
    import contextlib

    import concourse.bass as bass
    import concourse.mybir as mybir
    from concourse import library_config

    sched = plan["sched"]
    chunk = plan["chunk"]
    group = plan["group"]
    n_chunks = plan["n_chunks"]
    n_grp = len(sched)
    n_win = n_grp * group
    ic = plan["idxbuf"].shape[2]
    sc = plan["segbuf"].shape[2]

    # static per-group geometry
    grp_slots = [sum(ns for _, _, ns, _ in gs) for gs in sched]  # slots per group
    smax = max(grp_slots)
    # per (g): matmuls completed after group g (= slots)
    mm_after_grp = np.cumsum(grp_slots)
    # per window: matmul count = sum over chunks of its per-window slots
    win_slots = []
    for g in range(n_grp):
        for wl in range(group):
            win_slots.append(sum(gs[3][wl] for gs in sched[g]))
    mm_after_win = np.cumsum(win_slots)

    nc = bass.Bass(num_swdge_queues=4, dynamic_dma_scratch_size=16384)
    wz = nc.declare_dram_parameter("wz", [n_rows, d], mybir.dt.float32, isOutput=False)
    idx = nc.declare_dram_parameter("idx", [P, ic], mybir.dt.int16, isOutput=False)
    seg = nc.declare_dram_parameter("seg", [P, sc], mybir.dt.float32, isOutput=False)
    iota = nc.declare_dram_parameter("iota", [P, P], mybir.dt.float32, isOutput=False)
    out = nc.declare_dram_parameter(
        "out", [n_win * P, d], mybir.dt.float32, isOutput=True
    )
    wzb = nc.dram_tensor("wzb", [n_rows, d], mybir.dt.bfloat16)

    with contextlib.ExitStack() as ctx:
        idx_sb = ctx.enter_context(nc.sbuf_tensor([P, ic], mybir.dt.int16))
        seg_sb = ctx.enter_context(nc.sbuf_tensor([P, sc], mybir.dt.float32))
        iota_sb = ctx.enter_context(nc.sbuf_tensor([P, P], mybir.dt.float32))
        gbuf = ctx.enter_context(
            nc.sbuf_tensor([P, g_bufs * smax * d], mybir.dt.bfloat16)
        )
        ohbuf = ctx.enter_context(nc.sbuf_tensor([P, oh_bufs * P], mybir.dt.bfloat16))
        obuf = ctx.enter_context(nc.sbuf_tensor([P, o_bufs * d], mybir.dt.float32))
        psums = [
            ctx.enter_context(nc.psum_tensor(f"ps{i}", [P, d], mybir.dt.float32))
            for i in range(p_bufs)
        ]
        in_sem = ctx.enter_context(nc.semaphore("in_sem"))
        zsem = ctx.enter_context(nc.semaphore("zsem"))
        conv_sems = [
            ctx.enter_context(nc.semaphore(f"conv{c}")) for c in range(n_chunks)
        ]
        gsems = [ctx.enter_context(nc.semaphore(f"gsem{c}")) for c in range(n_chunks)]
        ohsem = ctx.enter_context(nc.semaphore("ohsem"))
        mmsem = ctx.enter_context(nc.semaphore("mmsem"))
        csem = ctx.enter_context(nc.semaphore("csem"))
        ssems = [ctx.enter_context(nc.semaphore(f"ssem{i}")) for i in range(o_bufs)]
        block = ctx.enter_context(nc.Block())

        @block.sync
        def _(sync):
            sync.dma_start(idx_sb[:], idx[:]).then_inc(in_sem, 16)
            sync.dma_start(seg_sb[:], seg[:]).then_inc(in_sem, 16)
            sync.dma_start(iota_sb[:], iota[:]).then_inc(in_sem, 16)
            for w in range(n_win):
                sync.wait_ge(csem, w + 1)
                sync.dma_start(
                    out[w * P : (w + 1) * P, :],
                    obuf[:, (w % o_bufs) * d : (w % o_bufs + 1) * d],
                ).then_inc(ssems[w % o_bufs], 16)
            for lane in range(o_bufs):
                n_l = len(range(lane, n_win, o_bufs))
                if n_l:
                    sync.wait_ge(ssems[lane], 16 * n_l)

        @block.gpsimd
        def _(g):
            g.load_library(library_config.mlp)
            g.memset(gbuf[:], 0.0).then_inc(zsem, 1)
            # f32 -> bf16 table conversion, one cast-DMA per chunk, spread
            # across SWDGE queues so conversions drain in parallel
            for c in range(n_chunks):
                lo, hi = c * chunk, min((c + 1) * chunk, n_rows)
                inst = g.dma_start(
                    wzb[lo:hi, :], wz[lo:hi, :], single_packet=False
                ).then_inc(conv_sems[c], 16)
                if c % 4:
                    inst.ins.queue = f"qPoolDynamic{c % 4}"
            g.wait_ge(zsem, 1)
            g.wait_ge(in_sem, 48)
            reg_ctx = g.register("ni_reg")
            ni = reg_ctx.__enter__()
            icol = 0
            for gi in range(n_grp):
                if gi >= g_bufs:
                    g.wait_ge(mmsem, int(mm_after_grp[gi - g_bufs]))
                base = (gi % g_bufs) * (smax * d)
                sofs = 0
                for c, num, n_slots, _ in sched[gi]:
                    if gi == 0:
                        g.wait_ge(conv_sems[c], 16)
                    g.reg_mov(ni, num)
                    g.dma_gather(
                        out_ap=gbuf[
                            :, base + sofs * d : base + (sofs + n_slots) * d
                        ].rearrange("p (s e) -> p s e", e=d),
                        in_ap=wzb[c * chunk : min((c + 1) * chunk, n_rows), :],
                        idxs_ap=idx_sb[:, icol : icol + num // 16],
                        num_idxs=num,
                        num_idxs_reg=ni,
                        elem_size=d,
                        single_packet=single_packet,
                        queue_num=c % 4,
                    ).then_inc(gsems[c], 16)
                    icol += num // 16
                    sofs += n_slots

        @block.vector
        def _(v):
            v.wait_ge(in_sem, 48)
            j = 0
            for gi in range(n_grp):
                for s in range(grp_slots[gi]):
                    if j >= oh_bufs:
                        v.wait_ge(mmsem, j - oh_bufs + 1)
                    v.tensor_tensor(
                        out=ohbuf[:, (j % oh_bufs) * P : (j % oh_bufs + 1) * P],
                        in0=seg_sb[:, j : j + 1].to_broadcast([P, P]),
                        in1=iota_sb[:],
                        op=mybir.AluOpType.is_equal,
                    ).then_inc(ohsem, 1)
                    j += 1

        @block.tensor
        def _(pe):
            pe.wait_ge(zsem, 1)
            j = 0  # matmul index in consumption order (= ohbuf ring index)
            for gi in range(n_grp):
                base = (gi % g_bufs) * (smax * d)
                for c, num, n_slots, wslots in sched[gi]:
                    pe.wait_ge(gsems[c], 16 * (gi + 1))
                # per (chunk, window-local): slot offset in the gather stream
                sec_ofs = []  # [chunk][window-local] -> slot offset
                so = 0
                for c, num, n_slots, wslots in sched[gi]:
                    offs = []
                    for wl in range(group):
                        offs.append(so)
                        so += wslots[wl]
                    sec_ofs.append(offs)
                for wl in range(group):
                    wg = gi * group + wl
                    if wg >= p_bufs:
                        pe.wait_ge(csem, wg - p_bufs + 1)
                    ns_w = win_slots[wg]
                    si = 0
                    for ci, (c, num, n_slots, wslots) in enumerate(sched[gi]):
                        for s in range(wslots[wl]):
                            slot = sec_ofs[ci][wl] + s
                            pe.wait_ge(ohsem, j + 1)
                            pe.matmul(
                                psums[wg % p_bufs][:],
                                lhsT=ohbuf[
                                    :, (j % oh_bufs) * P : (j % oh_bufs + 1) * P
                                ],
                                rhs=gbuf[:, base + slot * d : base + (slot + 1) * d],
                                start=(si == 0),
                                stop=(si == ns_w - 1),
                            ).then_inc(mmsem, 1)
                            si += 1
                            j += 1

        @block.scalar
        def _(a):
            for w in range(n_win):
                a.wait_ge(mmsem, int(mm_after_win[w]))
                if w >= o_bufs:
                    wp = w - o_bufs
                    a.wait_ge(ssems[wp % o_bufs], 16 * (wp // o_bufs + 1))
                a.copy(
                    obuf[:, (w % o_bufs) * d : (w % o_bufs + 1) * d],
                    psums[w % p_bufs][:],
                ).then_inc(csem, 1)

    return nc


def _build_program4(
    n_rows, d, plan, g_bufs=6, oh_bufs=12, p_bufs=4, o_bufs=6
):
    """v4: per-(window,chunk) 16-aligned bf16 gathers + one-hot matmul pooling.

    Same schedule/packing as _build_program2 (minimal descriptor count; slot
    tails hold stale data dropped via seg=-1 one-hot columns), but the table
    is converted once to bf16 in DRAM (SWDGE cast-DMA) so gathers move half
    the bytes and the pooling matmuls are single-pass bf16.  Chunk c's
    gathers ride SWDGE queue c so a window's four calls drain in parallel.
    """
    import contextlib

    import concourse.bass as bass
    import concourse.mybir as mybir
    from concourse import library_config

    sched = plan["sched"]
    chunk = plan["chunk"]
    n_win = len(sched)
    n_chunks = max(c for ws in sched for c, _, _ in ws) + 1
    ic = plan["idxbuf"].shape[2]
    sc = plan["segbuf"].shape[2]

    slots_per_win = [sum(ns for _, _, ns in ws) for ws in sched]
    g_per_win = [len(ws) for ws in sched]
    smax = max(slots_per_win)
    mm_after = np.cumsum(slots_per_win)
    # per (w, c): cumulative gather count on chunk c's queue after window w
    gcount = np.zeros((n_win, n_chunks), np.int64)
    run = [0] * n_chunks
    for w in range(n_win):
        for c, _, _ in sched[w]:
            run[c] += 1
        gcount[w] = run

    nc = bass.Bass(num_swdge_queues=4)
    wz = nc.declare_dram_parameter("wz", [n_rows, d], mybir.dt.float32, isOutput=False)
    idx = nc.declare_dram_parameter("idx", [P, ic], mybir.dt.int16, isOutput=False)
    seg = nc.declare_dram_parameter("seg", [P, sc], mybir.dt.float32, isOutput=False)
    iota = nc.declare_dram_parameter("iota", [P, P], mybir.dt.float32, isOutput=False)
    out = nc.declare_dram_parameter(
        "out", [n_win * P, d], mybir.dt.float32, isOutput=True
    )
    wzb = nc.dram_tensor("wzb", [n_rows, d], mybir.dt.bfloat16)

    with contextlib.ExitStack() as ctx:
        idx_sb = ctx.enter_context(nc.sbuf_tensor([P, ic], mybir.dt.int16))
        seg_sb = ctx.enter_context(nc.sbuf_tensor([P, sc], mybir.dt.float32))
        iota_sb = ctx.enter_context(nc.sbuf_tensor([P, P], mybir.dt.float32))
        gbuf = ctx.enter_context(
            nc.sbuf_tensor([P, g_bufs * smax * d], mybir.dt.bfloat16)
        )
        ohbuf = ctx.enter_context(nc.sbuf_tensor([P, oh_bufs * P], mybir.dt.bfloat16))
        obuf = ctx.enter_context(nc.sbuf_tensor([P, o_bufs * d], mybir.dt.float32))
        psums = [
            ctx.enter_context(nc.psum_tensor(f"ps{i}", [P, d], mybir.dt.float32))
            for i in range(p_bufs)
        ]
        in_sem = ctx.enter_context(nc.semaphore("in_sem"))
        zsem = ctx.enter_context(nc.semaphore("zsem"))
        conv_sems = [
            ctx.enter_context(nc.semaphore(f"conv{c}")) for c in range(n_chunks)
        ]
        gsems = [ctx.enter_context(nc.semaphore(f"gsem{c}")) for c in range(n_chunks)]
        ohsem = ctx.enter_context(nc.semaphore("ohsem"))
        mmsem = ctx.enter_context(nc.semaphore("mmsem"))
        csem = ctx.enter_context(nc.semaphore("csem"))
        ssems = [ctx.enter_context(nc.semaphore(f"ssem{i}")) for i in range(o_bufs)]
        block = ctx.enter_context(nc.Block())

        @block.sync
        def _(sync):
            sync.dma_start(idx_sb[:], idx[:]).then_inc(in_sem, 16)
            sync.dma_start(seg_sb[:], seg[:]).then_inc(in_sem, 16)
            sync.dma_start(iota_sb[:], iota[:]).then_inc(in_sem, 16)
            for w in range(n_win):
                sync.wait_ge(csem, w + 1)
                sync.dma_start(
                    out[w * P : (w + 1) * P, :],
                    obuf[:, (w % o_bufs) * d : (w % o_bufs + 1) * d],
                ).then_inc(ssems[w % o_bufs], 16)
            for lane in range(o_bufs):
                n_l = len(range(lane, n_win, o_bufs))
                if n_l:
                    sync.wait_ge(ssems[lane], 16 * n_l)

        @block.gpsimd
        def _(g):
            g.load_library(library_config.mlp)
            g.memset(gbuf[:], 0.0).then_inc(zsem, 1)
            # convert chunk c on SWDGE queue c%4 so conversions drain in
            # parallel and chunk-c gathers (same queue) queue right behind
            # their own chunk's conversion only.
            for c in range(n_chunks):
                lo, hi = c * chunk, min((c + 1) * chunk, n_rows)
                inst = g.dma_start(wzb[lo:hi, :], wz[lo:hi, :]).then_inc(
                    conv_sems[c], 16
                )
                if c % 4:
                    inst.ins.queue = f"qPoolDynamic{c % 4}"
            g.wait_ge(zsem, 1)
            g.wait_ge(in_sem, 48)
            reg_ctx = g.register("ni_reg")
            ni = reg_ctx.__enter__()
            icol = 0
            waited = set()
            for w in range(n_win):
                if w >= g_bufs:
                    g.wait_ge(mmsem, int(mm_after[w - g_bufs]))
                base = (w % g_bufs) * (smax * d)
                sofs = 0
                for c, p16, n_slots in sched[w]:
                    if c not in waited:
                        g.wait_ge(conv_sems[c], 16)
                        waited.add(c)
                    g.reg_mov(ni, p16)
                    g.dma_gather(
                        out_ap=gbuf[
                            :, base + sofs * d : base + (sofs + n_slots) * d
                        ].rearrange("p (s e) -> p s e", e=d),
                        in_ap=wzb[c * chunk : min((c + 1) * chunk, n_rows), :],
                        idxs_ap=idx_sb[:, icol : icol + p16 // 16],
                        num_idxs=p16,
                        num_idxs_reg=ni,
                        elem_size=d,
                        single_packet=False,
                        queue_num=c % 4,
                    ).then_inc(gsems[c], 16)
                    icol += p16 // 16
                    sofs += n_slots

        @block.vector
        def _(v):
            v.wait_ge(in_sem, 48)
            j = 0
            for w in range(n_win):
                for s in range(slots_per_win[w]):
                    if j >= oh_bufs:
                        v.wait_ge(mmsem, j - oh_bufs + 1)
                    v.tensor_tensor(
                        out=ohbuf[:, (j % oh_bufs) * P : (j % oh_bufs + 1) * P],
                        in0=seg_sb[:, j : j + 1].to_broadcast([P, P]),
                        in1=iota_sb[:],
                        op=mybir.AluOpType.is_equal,
                    ).then_inc(ohsem, 1)
                    j += 1

        @block.tensor
        def _(pe):
            pe.wait_ge(zsem, 1)
            j = 0
            for w in range(n_win):
                base = (w % g_bufs) * (smax * d)
                for c, _, _ in sched[w]:
                    pe.wait_ge(gsems[c], 16 * int(gcount[w][c]))
                if w >= p_bufs:
                    pe.wait_ge(csem, w - p_bufs + 1)
                ns = slots_per_win[w]
                for s in range(ns):
                    pe.wait_ge(ohsem, j + 1)
                    pe.matmul(
                        psums[w % p_bufs][:],
                        lhsT=ohbuf[:, (j % oh_bufs) * P : (j % oh_bufs + 1) * P],
                        rhs=gbuf[:, base + s * d : base + (s + 1) * d],
                        start=(s == 0),
                        stop=(s == ns - 1),
                    ).then_inc(mmsem, 1)
                    j += 1

        @block.scalar
        def _(a):
            for w in range(n_win):
                a.wait_ge(mmsem, int(mm_after[w]))
                if w >= o_bufs:
                    wp = w - o_bufs
                    a.wait_ge(ssems[wp % o_bufs], 16 * (wp // o_bufs + 1))
                a.copy(
                    obuf[:, (w % o_bufs) * d : (w % o_bufs + 1) * d],
                    psums[w % p_bufs][:],
                ).then_inc(csem, 1)

    return nc


def _build_program5(n_rows, d, plan, g_bufs=4, oh_bufs=6, p_bufs=2, o_bufs=4):
    """v5: exact _build_program2 skeleton, but the table is converted once to
    bf16 in DRAM (SWDGE cast-DMA prologue) and the gathers/one-hots/matmuls
    run in bf16 (single-pass PE)."""
    import contextlib

    import concourse.bass as bass
    import concourse.mybir as mybir
    from concourse import library_config

    sched = plan["sched"]
    chunk = plan["chunk"]
    n_win = len(sched)
    n_chunks = max(c for ws in sched for c, _, _ in ws) + 1
    ic = plan["idxbuf"].shape[2]
    sc = plan["segbuf"].shape[2]

    slots_per_win = [sum(ns for _, _, ns in ws) for ws in sched]
    g_per_win = [len(ws) for ws in sched]
    smax = max(slots_per_win)
    mm_after = np.cumsum(slots_per_win)
    lane_after = {}
    lane_tot = [0] * g_bufs
    for w in range(n_win):
        lane_tot[w % g_bufs] += 16 * g_per_win[w]
        lane_after[w] = lane_tot[w % g_bufs]

    nc = bass.Bass(num_swdge_queues=4)
    wz = nc.declare_dram_parameter("wz", [n_rows, d], mybir.dt.float32, isOutput=False)
    idx = nc.declare_dram_parameter("idx", [P, ic], mybir.dt.int16, isOutput=False)
    seg = nc.declare_dram_parameter("seg", [P, sc], mybir.dt.float32, isOutput=False)
    iota = nc.declare_dram_parameter("iota", [P, P], mybir.dt.float32, isOutput=False)
    out = nc.declare_dram_parameter(
        "out", [n_win * P, d], mybir.dt.float32, isOutput=True
    )
    wzb = nc.dram_tensor("wzb", [n_rows, d], mybir.dt.bfloat16)

    with contextlib.ExitStack() as ctx:
        idx_sb = ctx.enter_context(nc.sbuf_tensor([P, ic], mybir.dt.int16))
        seg_sb = ctx.enter_context(nc.sbuf_tensor([P, sc], mybir.dt.float32))
        iota_sb = ctx.enter_context(nc.sbuf_tensor([P, P], mybir.dt.float32))
        gbuf = ctx.enter_context(
            nc.sbuf_tensor([P, g_bufs * smax * d], mybir.dt.bfloat16)
        )
        ohbuf = ctx.enter_context(nc.sbuf_tensor([P, oh_bufs * P], mybir.dt.bfloat16))
        obuf = ctx.enter_context(nc.sbuf_tensor([P, o_bufs * d], mybir.dt.float32))
        psums = [
            ctx.enter_context(nc.psum_tensor(f"ps{i}", [P, d], mybir.dt.float32))
            for i in range(p_bufs)
        ]
        in_sem = ctx.enter_context(nc.semaphore("in_sem"))
        zsem = ctx.enter_context(nc.semaphore("zsem"))
        conv_sem = ctx.enter_context(nc.semaphore("conv_sem"))
        gsems = [ctx.enter_context(nc.semaphore(f"gsem{i}")) for i in range(g_bufs)]
        ohsem = ctx.enter_context(nc.semaphore("ohsem"))
        mmsem = ctx.enter_context(nc.semaphore("mmsem"))
        csem = ctx.enter_context(nc.semaphore("csem"))
        ssems = [ctx.enter_context(nc.semaphore(f"ssem{i}")) for i in range(o_bufs)]
        block = ctx.enter_context(nc.Block())

        @block.sync
        def _(sync):
            sync.dma_start(idx_sb[:], idx[:]).then_inc(in_sem, 16)
            sync.dma_start(seg_sb[:], seg[:]).then_inc(in_sem, 16)
            sync.dma_start(iota_sb[:], iota[:]).then_inc(in_sem, 16)
            for w in range(n_win):
                sync.wait_ge(csem, w + 1)
                sync.dma_start(
                    out[w * P : (w + 1) * P, :],
                    obuf[:, (w % o_bufs) * d : (w % o_bufs + 1) * d],
                ).then_inc(ssems[w % o_bufs], 16)
            for lane in range(o_bufs):
                n_l = len(range(lane, n_win, o_bufs))
                if n_l:
                    sync.wait_ge(ssems[lane], 16 * n_l)

        @block.gpsimd
        def _(g):
            g.load_library(library_config.mlp)
            g.memset(gbuf[:], 0.0).then_inc(zsem, 1)
            for c in range(n_chunks):
                lo, hi = c * chunk, min((c + 1) * chunk, n_rows)
                g.dma_start(wzb[lo:hi, :], wz[lo:hi, :]).then_inc(conv_sem, 16)
            g.wait_ge(zsem, 1)
            g.wait_ge(in_sem, 48)
            g.wait_ge(conv_sem, 16 * n_chunks)
            reg_ctx = g.register("ni_reg")
            ni = reg_ctx.__enter__()
            icol = 0
            for w in range(n_win):
                if w >= g_bufs:
                    g.wait_ge(mmsem, int(mm_after[w - g_bufs]))
                base = (w % g_bufs) * (smax * d)
                sofs = 0
                for c, p16, n_slots in sched[w]:
                    g.reg_mov(ni, p16)
                    g.dma_gather(
                        out_ap=gbuf[
                            :, base + sofs * d : base + (sofs + n_slots) * d
                        ].rearrange("p (s e) -> p s e", e=d),
                        in_ap=wzb[c * chunk : min((c + 1) * chunk, n_rows), :],
                        idxs_ap=idx_sb[:, icol : icol + p16 // 16],
                        num_idxs=p16,
                        num_idxs_reg=ni,
                        elem_size=d,
                        single_packet=False,
                        queue_num=w % g_bufs % 4,
                    ).then_inc(gsems[w % g_bufs], 16)
                    icol += p16 // 16
                    sofs += n_slots

        @block.vector
        def _(v):
            v.wait_ge(in_sem, 48)
            j = 0
            for w in range(n_win):
                for s in range(slots_per_win[w]):
                    if j >= oh_bufs:
                        v.wait_ge(mmsem, j - oh_bufs + 1)
                    v.tensor_tensor(
                        out=ohbuf[:, (j % oh_bufs) * P : (j % oh_bufs + 1) * P],
                        in0=seg_sb[:, j : j + 1].to_broadcast([P, P]),
                        in1=iota_sb[:],
                        op=mybir.AluOpType.is_equal,
                    ).then_inc(ohsem, 1)
                    j += 1

        @block.tensor
        def _(pe):
            pe.wait_ge(zsem, 1)
            j = 0
            for w in range(n_win):
                base = (w % g_bufs) * (smax * d)
                pe.wait_ge(gsems[w % g_bufs], lane_after[w])
                if w >= p_bufs:
                    pe.wait_ge(csem, w - p_bufs + 1)
                ns = slots_per_win[w]
                for s in range(ns):
                    pe.wait_ge(ohsem, j + 1)
                    pe.matmul(
                        psums[w % p_bufs][:],
                        lhsT=ohbuf[:, (j % oh_bufs) * P : (j % oh_bufs + 1) * P],
                        rhs=gbuf[:, base + s * d : base + (s + 1) * d],
                        start=(s == 0),
                        stop=(s == ns - 1),
                    ).then_inc(mmsem, 1)
                    j += 1

        @block.scalar
        def _(a):
            for w in range(n_win):
                a.wait_ge(mmsem, int(mm_after[w]))
                if w >= o_bufs:
                    wp = w - o_bufs
                    a.wait_ge(ssems[wp % o_bufs], 16 * (wp // o_bufs + 1))
                a.copy(
                    obuf[:, (w % o_bufs) * d : (w % o_bufs + 1) * d],
                    psums[w % p_bufs][:],
                ).then_inc(csem, 1)

    return nc


def _run(weights, indices, offsets, trace=False, v2=True, chunk=None, v3=True):
    from concourse import mybir
    from concourse.bass_utils import run_bass_kernel_spmd

    weights = np.ascontiguousarray(np.asarray(weights), dtype=np.float32)
    t, n, d = weights.shape

    if v3:
        try:
            chunk3 = chunk
            if chunk3 is None:
                chunk3 = -(-n // max(1, -(-n // 32767)))
            plan = _plan8(indices, offsets, n, chunk=chunk3, group=8)
            if plan is None:
                raise ValueError("v8 fast path needs fixed-size bags")
            nc = _build_program8(n, d, plan)
            mybir.codegen_inst_isa_subclasses(nc)
            in_maps = [
                {
                    "wz": weights[i],
                    "idx": np.ascontiguousarray(plan["idxbuf"][i]),
                    "seg": np.ascontiguousarray(plan["segbuf"][i]),
                    "iota": plan["iota"],
                }
                for i in range(t)
            ]
        except Exception:
            in_maps = None
        if in_maps is not None:
            res = run_bass_kernel_spmd(nc, in_maps, list(range(t)), trace=trace)
            out = np.stack([res.results[i]["out"] for i in range(t)], axis=0)
            return out, res

    if v2:
        if chunk is None:
            chunk = -(-n // max(1, -(-n // 32767)))  # even chunks, each <= 32767
        plan = _plan2(indices, offsets, n, chunk=chunk)
        nc = _build_program2(n, d, plan)
        mybir.codegen_inst_isa_subclasses(nc)
        in_maps = [
            {
                "wz": weights[i],
                "idx": np.ascontiguousarray(plan["idxbuf"][i]),
                "seg": np.ascontiguousarray(plan["segbuf"][i]),
                "iota": plan["iota"],
            }
            for i in range(t)
        ]
    else:
        idxbufs, lws, col_ofs, sum_l, need_pad = _plan(indices, offsets, n)
        n_win = np.asarray(offsets).shape[1] // P
        if need_pad:
            wz = np.concatenate([weights, np.zeros((t, 1, d), np.float32)], axis=1)
        else:
            wz = weights
        nc = _build_program(wz.shape[1], d, n_win, lws, col_ofs, sum_l)
        in_maps = [
            {"wz": wz[i], "idx": np.ascontiguousarray(idxbufs[i])} for i in range(t)
        ]
    res = run_bass_kernel_spmd(nc, in_maps, list(range(t)), trace=trace)
    out = np.stack([res.results[i]["out"] for i in range(t)], axis=0)
    return out, res


def kernel(weights, indices, offsets):
    out, _ = _run(weights, indices, offsets)
    return out

